# revision 8
# baseline (speedup 1.0000x reference)
"""Trainium2 Bass kernel for a dense transformer AttentionBlock.

Problem (fixed shapes): B=4, S=2048, D=512, H=8 heads (HD=64), FFN hidden 2048.
  qkv = x @ in_proj_w.T + b ; attn = softmax(q k^T / sqrt(64)) ; ctx = attn @ v
  x1 = LN(x + ctx @ out_w.T + out_b) ; out = LN(x1 + gelu(x1 @ w1.T + b1) @ w2.T + b2)

Sharding: 8 cores, zero collectives. Core c handles batch b=c//2, sequence half
h=c%2 (1024 query tokens). K/V are computed redundantly for the full 2048-token
sequence of the batch on both cores of a pair. One SPMD program for all cores:
for odd cores the host rolls x^T by -1024 columns so the core's own query
tokens always sit at columns [0,1024) (k-token order is irrelevant to softmax).

The kernel is scheduled around the softmax-exp wall: exp of the 8*2048*1024
score matrix runs only on the Activation engine (~128 x 1us instructions) and
is the binding resource of the attention phase, so everything else is arranged
to hide beneath it:
  - minimal prefix (Q head-pair 0, K half 0) starts the first exp ~12us in;
    V / remaining Q / K projections ride the PE queue interleaved under the
    head-pair-0 exp stream, with per-head-pair softmax normalization spaced
    into the NEXT pair's window so boundary scores never wait on a DVE chain.
  - an idle PE drops to 0.65GHz and needs ~3us to re-ramp to 2.4GHz, so the
    schedule never lets PE drain: throwaway warm-up matmuls cover the initial
    DMA wait.
Dtypes: weights/activations feeding matmuls are host-cast bf16 (halves DMA +
SBUF; PE rate is identical to f32r). Three paths run fp8e4m3 with DoubleRow
matmuls (2 contraction rows/cell/cycle): the V projection, attn @ V (exp
emits fp8 weights; the row-sum — via one-hot columns appended to V, landing
on psum rows 64:72 — is computed from the SAME quantized weights, so softmax
renormalization cancels the quantization almost exactly: measured +0e0 vs
bf16), and FFN1 (x2/w1, ~9e-3 rel err, the dominant error term). FFN2 stays
bf16: both FFN layers in fp8 measured 1.47e-2 — too close to the 2e-2 gate.
The residual/LayerNorm path stays f32.

Timing support: KERNEL_REPS>1 wraps the body in a tc.For_i HARDWARE loop so
program size stays constant while device work scales linearly; test.py
extracts true device time as the slope of wall time vs reps (per-call axon
dispatch overhead is ~2-3s, noisy, and grows with program size, so a
Python-unrolled rep loop measures the host, not the kernel).
"""

import os
import numpy as np
import ml_dtypes
from contextlib import ExitStack

import concourse.bass as bass
import concourse.mybir as mybir
import concourse.tile as tile
from concourse import bacc
from concourse.bass_utils import run_bass_kernel_spmd

F32 = mybir.dt.float32
F32R = mybir.dt.float32r
BF16 = mybir.dt.bfloat16
FP8 = mybir.dt.float8e4
PM = mybir.MatmulPerfMode
AF = mybir.ActivationFunctionType
OP = mybir.AluOpType

B, S, D, H = 4, 2048, 512, 8
HD = D // H          # 64
F = 4 * D            # 2048
SQ = S // 2          # 1024 own query tokens per core
EPS = 1e-5
N_CORES = 8

# vext: per head 72 columns = [v(64) | 8 filler]; ones at col 72*h + 64 + h
VW = 72
VEXT_W = H * VW      # 576


def _emit(nc, flags):
    """Emit the whole per-core program. flags: dict of bools for optional ops.
    KERNEL_STOP_AFTER in {qkv, attn, ln1, ffn1} truncates for cost analysis.
    KERNEL_REPS>1 wraps the body in a tc.For_i HARDWARE loop: the program size
    stays constant while device work scales linearly, so wall-time deltas
    between two reps values isolate true device execution time (per-call axon
    dispatch overhead is large, noisy, and scales with program size — a
    Python-unrolled rep loop measures that overhead, not the kernel)."""
    stop_after = os.environ.get("KERNEL_STOP_AFTER", "")
    reps = int(os.environ.get("KERNEL_REPS", "1"))
    # ---- DRAM parameters ----
    xT_d = nc.declare_dram_parameter("xT", [D, S], BF16, isOutput=False)
    xown_d = nc.declare_dram_parameter("x_own", [SQ, D], F32R, isOutput=False)
    wqkvT_d = nc.declare_dram_parameter("wqkvT", [D, 2 * D], BF16, isOutput=False)
    xT8_d = nc.declare_dram_parameter("xT8", [D, S], FP8, isOutput=False)
    wv8_d = nc.declare_dram_parameter("wv8", [D, D], FP8, isOutput=False)
    bqkv_d = nc.declare_dram_parameter("bqkv_pp", [128, 12], F32, isOutput=False)
    woutT_d = nc.declare_dram_parameter("woutT", [D, D], BF16, isOutput=False)
    w1T_d = nc.declare_dram_parameter("w1T", [D, F], FP8, isOutput=False)
    b1_d = nc.declare_dram_parameter("b1_pp", [128, 16], F32, isOutput=False)
    w2T_d = nc.declare_dram_parameter("w2T", [F, D], BF16, isOutput=False)
    assign_d = nc.declare_dram_parameter("assign", [8, 4, 128], F32R, isOutput=False)
    ident_d = nc.declare_dram_parameter("ident", [128, 128], F32R, isOutput=False)
    vecs_d = nc.declare_dram_parameter("vecs", [7, D], F32, isOutput=False)
    out_d = nc.declare_dram_parameter("out", [SQ, D], F32, isOutput=True)

    VEC_ROW = {"bv": 0, "bout": 1, "b2": 2, "g1": 3, "bt1": 4, "g2": 5, "bt2": 6}

    dma = nc.gpsimd.dma_start      # stores / misc (SWDGE on Pool)
    ldma = nc.sync.dma_start       # loads (HWDGE issued from idle SP engine)

    def bcast_row(pool, name, row):
        t = pool.tile([128, D], F32, tag=f"bc_{name}", name=f"bc_{name}")
        src = vecs_d[row]  # (D,)
        src_b = bass.AP(tensor=src.tensor, offset=src.offset,
                        ap=[[0, 128]] + list(src.ap))
        dma(out=t[:], in_=src_b)
        return t

    def body(tc):
        with ExitStack() as es:
            persist = es.enter_context(tc.tile_pool(name="persist", bufs=1))
            work = es.enter_context(tc.tile_pool(name="work", bufs=2))
            psum = es.enter_context(tc.tile_pool(name="psum", bufs=2, space="PSUM"))
            xo = es.enter_context(tc.tile_pool(name="xo", bufs=1))
            shr = es.enter_context(tc.tile_pool(name="shr", bufs=1))
            w1p = es.enter_context(tc.tile_pool(name="w1p", bufs=1))

            def ps_big(nm):
                # 4-bank ctx accumulator (one buffer)
                return psum.tile([128, 2048], F32, tag="c", name=nm, bufs=1)

            def ps_tile(nm, dt=F32):
                # 2-bank double-buffered working psum
                return psum.tile([128, 1024], dt, tag="s", name=nm, bufs=2)

            # ---- tiny persistent tensors (cheap DMAs; big loads below) ----
            bqkv_sb = persist.tile([128, 12], F32, name="bqkv_sb")
            ldma(out=bqkv_sb[:], in_=bqkv_d[:])
            b1_sb = persist.tile([128, 16], F32, name="b1_sb")
            ldma(out=b1_sb[:], in_=b1_d[:])
            eps_sb = persist.tile([128, 1], F32, name="eps_sb")
            nc.vector.memset(eps_sb[:], EPS)
            bc = {}
            for nm in ("bv", "bout", "b2", "g1", "bt1", "g2", "bt2"):
                if flags[nm]:
                    bc[nm] = bcast_row(persist, nm, VEC_ROW[nm])
            # tiles whose loads are deferred off the critical DMA path
            woutT_sb = persist.tile([128, 4, D], BF16, name="woutT_sb")
            assign_sb = persist.tile([128, 4, 128], F32R, name="assign_sb")
            ident_sb = persist.tile([128, 128], F32R, name="ident_sb")
            xown_sb = xo.tile([128, 8, D], F32R, name="xown_sb")
            # shared feature-major activation tile: ctx^T, later x2^T
            actT_sb = shr.tile([128, 4, SQ], BF16, name="actT_sb")
            # FFN w1, prefetched during attention
            w1T_sb = w1p.tile([128, 4, F], FP8, name="w1T_sb")
            actT8_sb = w1p.tile([128, 4, SQ], FP8, name="actT8_sb")

            with tc.tile_pool(name="qk", bufs=1) as qk:
                qT_sb = qk.tile([128, 4, SQ], BF16, name="qT_sb")
                kT_sb = qk.tile([128, 4, S], BF16, name="kT_sb")
                vext_sb = qk.tile([128, 16, VEXT_W], FP8, name="vext_sb")
                # vext filler: zero cols 64:72 per head, then 1.0 at col 64+h
                # (row-sum one-hot). Pool memsets, no DMA traffic.
                vfill = vext_sb[:].rearrange("p t (h w) -> p t h w", w=VW)
                nc.gpsimd.memset(vfill[:, :, :, HD:VW], 0.0)
                for h in range(H):
                    nc.gpsimd.memset(vfill[:, :, h, HD + h:HD + h + 1], 1.0)

                rsum_sb = qk.tile([128, SQ], F32R, name="rsum_sb")  # 64:72
                with tc.tile_pool(name="pp", bufs=4) as pp, \
                     tc.tile_pool(name="pha", bufs=1) as pha:
                    xT_sb = pha.tile([128, 4, S], BF16, name="xT_sb")
                    wqkvT_sb = pha.tile([128, 4, 2 * D], BF16, name="wqkvT_sb")
                    xT8_sb = pha.tile([128, 4, S], FP8, name="xT8_sb")
                    wv8_sb = pha.tile([128, 4, D], FP8, name="wv8_sb")

                    # ---- load order = need order: Q needs xT cols 0:1024 +
                    # wqkv cols 0:512; K0 next; V cols next; rest deferred.
                    xT_src = xT_d.ap().rearrange("(c p) t -> p c t", p=128)
                    for tq in range(2):
                        ldma(out=xT_sb[:, :, 512 * tq:512 * tq + 512],
                             in_=xT_src[:, :, 512 * tq:512 * tq + 512])
                    for c in range(4):   # Q weight cols (first exp needs them)
                        ldma(out=wqkvT_sb[:, c, 0:512],
                             in_=wqkvT_d[128 * c:128 * c + 128, 0:512])
                    for c in range(4):   # K weight cols
                        ldma(out=wqkvT_sb[:, c, 512:1024],
                             in_=wqkvT_d[128 * c:128 * c + 128, 512:1024])
                    xT8_src = xT8_d.ap().rearrange("(c p) t -> p c t", p=128)
                    ldma(out=wv8_sb[:],
                         in_=wv8_d.ap().rearrange("(c p) m -> p c m", p=128))
                    for tq8 in range(2):
                        ldma(out=xT8_sb[:, :, 1024 * tq8:1024 * tq8 + 1024],
                             in_=xT8_src[:, :, 1024 * tq8:1024 * tq8 + 1024])
                    for tq in range(2, 4):
                        ldma(out=xT_sb[:, :, 512 * tq:512 * tq + 512],
                             in_=xT_src[:, :, 512 * tq:512 * tq + 512])
                    # off-critical-path loads (after the attention-feeding ones)
                    ldma(out=xown_sb[:],
                         in_=xown_d.ap().rearrange("(j p) d -> p j d", p=128))
                    ldma(out=woutT_sb[:],
                         in_=woutT_d.ap().rearrange("(c p) m -> p c m", p=128))
                    ldma(out=assign_sb[64:72, :, :], in_=assign_d[:])
                    ldma(out=ident_sb[:], in_=ident_d[:])
                    ldma(out=w1T_sb[:],
                         in_=w1T_d.ap().rearrange("(c p) m -> p c m", p=128))
                    if stop_after == "dma":
                        return

                    def emit_q(mi):
                        q_ps = ps_tile(f"q_ps{mi}")
                        for tj in range(2):
                            for c in range(4):
                                nc.tensor.matmul(
                                    q_ps[:, 512 * tj:512 * tj + 512],
                                    wqkvT_sb[:, c, 128 * mi:128 * mi + 128],
                                    xT_sb[:, c, 512 * tj:512 * tj + 512],
                                    start=(c == 0), stop=(c == 3))
                        nc.vector.tensor_scalar(
                            qT_sb[:, mi, :], q_ps[:],
                            bqkv_sb[:, mi:mi + 1], None, OP.add)

                    def emit_k_half(mi, kh):
                        # K^T for head-pair mi, token half kh; evac on DVE
                        # (keeps ACT free for softmax exp)
                        k_ps = ps_tile(f"k_ps{mi}_{kh}")
                        for tj in range(2):
                            for c in range(4):
                                nc.tensor.matmul(
                                    k_ps[:, 512 * tj:512 * tj + 512],
                                    wqkvT_sb[:, c,
                                             512 + 128 * mi:512 + 128 * mi + 128],
                                    xT_sb[:, c, 1024 * kh + 512 * tj:
                                          1024 * kh + 512 * tj + 512],
                                    start=(c == 0), stop=(c == 3))
                        nc.vector.tensor_scalar(
                            kT_sb[:, mi, 1024 * kh:1024 * kh + 1024],
                            k_ps[:], bqkv_sb[:, 4 + mi:5 + mi], None,
                            OP.add)

                    def emit_k(mi):
                        emit_k_half(mi, 0)
                        emit_k_half(mi, 1)

                    def emit_v(ti):
                        # fp8 DoubleRow: two 128-feature contraction chunks
                        # per instruction (V tolerates fp8 inputs well; the
                        # attention average washes element noise out)
                        v_ps = ps_tile(f"v_ps{ti}")
                        for cp in range(2):
                            nc.tensor.matmul(
                                v_ps[:, 0:512],
                                xT8_sb[:, 2 * cp:2 * cp + 2,
                                       128 * ti:128 * ti + 128],
                                wv8_sb[:, 2 * cp:2 * cp + 2, :],
                                start=(cp == 0), stop=(cp == 1),
                                perf_mode=PM.DoubleRow)
                        v_dst = vext_sb[:, ti, :].rearrange(
                            "p (h e) -> p h e", e=VW)[:, :, 0:HD]
                        v_src = v_ps[:, 0:512].rearrange("p (h e) -> p h e", e=HD)
                        if flags["bv"]:
                            nc.vector.tensor_tensor(
                                v_dst, v_src,
                                bc["bv"][:].rearrange("p (h e) -> p h e", e=HD),
                                OP.add)
                        else:
                            nc.vector.tensor_copy(v_dst, v_src)

                    def attn_step(hp, ktp, hh, c_ps, ppool):
                        # scores -> exp -> ctx for one (head-pair, k-tile
                        # PAIR, hh). exp emits fp8 attention weights for the
                        # two k-tiles side by side; ctx then contracts both
                        # in one DoubleRow matmul per 512-token column chunk
                        # (2 fp8 contraction rows per PE cell per cycle).
                        # Softmax renormalizes by the sum of the same fp8
                        # weights (one-hot V columns), so weight quantization
                        # largely cancels.
                        h = 2 * hp + hh
                        p_sb = ppool.tile([128, 2, 1024], FP8, tag="p",
                                          name=f"p{hp}_{ktp}_{hh}")
                        for ki in range(2):
                            kt = 2 * ktp + ki
                            s_ps = ps_tile(f"s_ps{hp}_{ktp}_{hh}_{ki}")
                            lhsT = kT_sb[64 * hh:64 * hh + 64, hp,
                                         128 * kt:128 * kt + 128]
                            for tj in range(2):
                                nc.tensor.matmul(
                                    s_ps[:, 512 * tj:512 * tj + 512],
                                    lhsT,
                                    qT_sb[64 * hh:64 * hh + 64, hp,
                                          512 * tj:512 * tj + 512],
                                    start=True, stop=True)
                            nc.scalar.activation(out=p_sb[:, ki, :],
                                                 in_=s_ps[:], func=AF.Exp)
                        lhsT = vext_sb[:].rearrange(
                            "p t w -> p t w")[:, 2 * ktp:2 * ktp + 2,
                                              VW * h:VW * h + VW]
                        for tj in range(2):
                            nc.tensor.matmul(
                                c_ps[0:VW, 1024 * hh + 512 * tj:
                                     1024 * hh + 512 * tj + 512],
                                lhsT,
                                p_sb[:, :, 512 * tj:512 * tj + 512],
                                start=(ktp == 0), stop=(ktp == 7),
                                perf_mode=PM.DoubleRow)

                    def evac_attn(hp, c_ps):
                        # ctx^T rows 0:64 -> actT. Head 2hp+hh's row-sum sits
                        # on psum row 64+2hp+hh of column half hh (one-hot V
                        # column; other heads' rows are zero there, so the
                        # aligned 8-row block accumulates cleanly).
                        for hh in range(2):
                            nc.vector.tensor_copy(
                                actT_sb[64 * hh:64 * hh + 64, hp, :],
                                c_ps[0:64, 1024 * hh:1024 * hh + SQ])
                            if hp == 0 and hh == 0:
                                nc.vector.tensor_copy(
                                    rsum_sb[64:72, :], c_ps[64:72, 0:SQ])
                            else:
                                nc.vector.tensor_tensor(
                                    rsum_sb[64:72, :], rsum_sb[64:72, :],
                                    c_ps[64:72, 1024 * hh:1024 * hh + SQ],
                                    OP.add)

                    # PE p-state warmup: the array clocks 0.65->2.4GHz
                    # only after ~3us of continuous work; run throwaway
                    # matmuls on a zeroed scratch tile while the first xT/w
                    # DMAs are in flight so Q/K/scores start at full clock.
                    scr = pha.tile([128, 512], BF16, name="scr_sb")
                    nc.gpsimd.memset(scr[:], 0.0)
                    for wu in range(8):
                        w_ps = ps_tile(f"wu{wu}")
                        nc.tensor.matmul(w_ps[:, 0:512], scr[:, 0:128],
                                         scr[:], start=True, stop=True)
                    # Minimal prefix before the first softmax exp: Q(mi0)
                    # and K(0, token-half 0) — everything else (Q1-3, K0h1,
                    # V, K1) rides the PE queue interleaved under hp0's exp
                    # wall. hp0's first 8 k-tiles live in token half 0.
                    emit_q(0)
                    emit_k_half(0, 0)
                    if stop_after == "qkv":
                        for mi in range(1, 4):
                            emit_q(mi)
                        emit_k_half(0, 1)
                        for ti in range(16):
                            emit_v(ti)
                        for mi in range(1, 4):
                            emit_k(mi)
                        return

                    def norm_hp(hp):
                        # per-head-pair softmax denominator. rsum rows for
                        # this hp are final after evac_attn(hp) (later adds
                        # only contribute zeros); stale rows of other pairs
                        # are masked by zeros in assign. The broadcast matmul
                        # runs on the RAW sums; the reciprocal is taken on
                        # the partition-0-aligned broadcast result (single
                        # partitions 65.. are not engine-addressable).
                        n_ps = ps_tile(f"n_ps{hp}")
                        for tj in range(2):
                            nc.tensor.matmul(
                                n_ps[:, 512 * tj:512 * tj + 512],
                                assign_sb[64:72, hp, :],
                                rsum_sb[64:72, 512 * tj:512 * tj + 512],
                                start=True, stop=True)
                        nrec = work.tile([128, SQ], F32, tag="nr",
                                         name=f"nrec{hp}")
                        nc.vector.reciprocal(nrec[:], n_ps[:, 0:SQ])
                        with nc.allow_low_precision(
                                reason="attention weights tolerate bf16"):
                            nc.vector.tensor_tensor(
                                actT_sb[:, hp, :], actT_sb[:, hp, :],
                                nrec[:], OP.mult)

                    c_ps0 = ps_big("c_ps0")
                    for ktp in range(8):
                        emit_v(2 * ktp)
                        emit_v(2 * ktp + 1)
                        attn_step(0, ktp, 0, c_ps0, pp)
                        attn_step(0, ktp, 1, c_ps0, pp)
                        if ktp < 3:
                            emit_q(ktp + 1)
                        elif ktp == 3:
                            emit_k_half(0, 1)
                        elif ktp in (4, 6):
                            emit_k_half(1, (ktp - 4) // 2)
                    evac_attn(0, c_ps0)
                    for hp in range(1, 4):
                        c_ps = ps_big(f"c_ps{hp}")
                        for ktp in range(8):
                            attn_step(hp, ktp, 0, c_ps, pp)
                            attn_step(hp, ktp, 1, c_ps, pp)
                            if hp < 3 and ktp in (2, 5):
                                emit_k_half(hp + 1, (ktp - 2) // 3)
                            elif ktp == 1:
                                norm_hp(hp - 1)
                        evac_attn(hp, c_ps)
                    norm_hp(3)
                    # keep the PE array clocked through the evac/normalize
                    # DVE chain (an idle PE drops to 0.65GHz and would crawl
                    # through the first attn-out matmuls while re-ramping)
                    for wu in range(8):
                        w_ps = ps_tile(f"wt{wu}")
                        nc.tensor.matmul(w_ps[:, 0:512], scr[:, 0:128],
                                         scr[:], start=True, stop=True)



            if stop_after == "attn":
                return
            # ---- attn_out (natural) + LN1 -> x2 (in place over x_own) ----
            def layer_norm(j, acc_ps, resid_ap, out_ap, pre_b, g, bt):
                z = work.tile([128, D], F32, tag="z", name=f"z{j}")
                nc.vector.tensor_tensor(z[:], resid_ap, acc_ps, OP.add)
                if pre_b is not None:
                    nc.vector.tensor_tensor(z[:], z[:], pre_b[:], OP.add)
                st = work.tile([128, 6], F32, tag="st", name=f"st{j}")
                nc.vector.bn_stats(out=st[:], in_=z[:])
                mv = work.tile([128, 2], F32, tag="mv", name=f"mv{j}")
                nc.vector.bn_aggr(out=mv[:], in_=st[:])
                sd = work.tile([128, 1], F32, tag="sd", name=f"sd{j}")
                nc.scalar.activation(out=sd[:], in_=mv[:, 1:2], func=AF.Sqrt,
                                     bias=eps_sb[:], scale=1.0)
                nc.vector.reciprocal(sd[:], sd[:])
                nc.vector.tensor_scalar(out_ap, z[:], mv[:, 0:1], sd[:],
                                        OP.subtract, OP.mult)
                if g is not None:
                    nc.vector.tensor_tensor(out_ap, out_ap, g[:], OP.mult)
                if bt is not None:
                    nc.vector.tensor_tensor(out_ap, out_ap, bt[:], OP.add)

            if True:
                for j in range(8):
                    a_ps = ps_tile(f"a_ps{j}")
                    for c in range(4):
                        nc.tensor.matmul(a_ps[:, 0:512],
                                         actT_sb[:, c, 128 * j:128 * j + 128],
                                         woutT_sb[:, c, :],
                                         start=(c == 0), stop=(c == 3))
                    layer_norm(j, a_ps[:, 0:512], xown_sb[:, j, :],
                               xown_sb[:, j, :],
                               bc.get("bout"), bc.get("g1"), bc.get("bt1"))

                if stop_after == "ln1":
                    return
                # ============ phase 3: FFN + LN2 ============
                # x2^T via PE transposes (into actT, reusing the ctx^T tile)
                for i in range(4):
                    t_ps = ps_tile(f"t_ps{i}", F32R)
                    for j in range(8):
                        nc.tensor.transpose(t_ps[:, 128 * j:128 * j + 128],
                                            xown_sb[:, j, 128 * i:128 * i + 128],
                                            ident_sb[:])
                    with nc.allow_low_precision(
                            reason="x2 feeds fp8 DoubleRow FFN matmuls"):
                        nc.vector.tensor_copy(actT8_sb[:, i, :], t_ps[:])

                with tc.tile_pool(name="hp_", bufs=1) as hpool, \
                     tc.tile_pool(name="w2p", bufs=1) as w2p:
                    w2T_sb = w2p.tile([128, 16, D], BF16, name="w2T_sb")
                    for c in range(0, 16, 4):
                        dma(out=w2T_sb[:, c:c + 4, :],
                            in_=w2T_d.ap().rearrange("(c p) m -> p c m",
                                                     p=128)[:, c:c + 4, :])
                    hT_sb = hpool.tile([128, 16, SQ], BF16, name="hT_sb")

                    def ffn1_m(m):
                        f_ps = ps_tile(f"f_ps{m}")
                        for tj in range(2):
                            for cp in range(2):
                                nc.tensor.matmul(
                                    f_ps[:, 512 * tj:512 * tj + 512],
                                    w1T_sb[:, 2 * cp:2 * cp + 2,
                                           128 * m:128 * m + 128],
                                    actT8_sb[:, 2 * cp:2 * cp + 2,
                                             512 * tj:512 * tj + 512],
                                    start=(cp == 0), stop=(cp == 1),
                                    perf_mode=PM.DoubleRow)
                        nc.scalar.activation(out=hT_sb[:, m, :], in_=f_ps[:],
                                             func=AF.Gelu,
                                             bias=b1_sb[:, m:m + 1], scale=1.0)

                    def ffn2_fc(y_ps, wave, fc):
                        for jj in range(4):
                            j = 4 * wave + jj
                            nc.tensor.matmul(
                                y_ps[:, 512 * jj:512 * jj + 512],
                                hT_sb[:, fc, 128 * j:128 * j + 128],
                                w2T_sb[:, fc, :],
                                start=(fc == 0), stop=(fc == 15))

                    def ln2_wave(y_ps, wave):
                        for jj in range(4):
                            j = 4 * wave + jj
                            o_sb = work.tile([128, D], F32, tag="o",
                                             name=f"o{j}")
                            layer_norm(8 + j, y_ps[:, 512 * jj:512 * jj + 512],
                                       xown_sb[:, j, :], o_sb[:],
                                       bc.get("b2"), bc.get("g2"),
                                       bc.get("bt2"))
                            dma(out=out_d[128 * j:128 * j + 128, :],
                                in_=o_sb[:])

                    for m in range(16):
                        ffn1_m(m)
                    if stop_after == "ffn1":
                        return
                    # plain per-token-block FFN2: each block's 16-chunk psum
                    # accumulation chases the gelu stream naturally (fc<m
                    # chunks run while later gelus are still in flight)
                    for j in range(8):
                        y_ps = ps_tile(f"y_ps{j}")
                        for fc in range(16):
                            nc.tensor.matmul(y_ps[:, 0:512],
                                             hT_sb[:, fc, 128 * j:128 * j + 128],
                                             w2T_sb[:, fc, :],
                                             start=(fc == 0), stop=(fc == 15))
                        o_sb = work.tile([128, D], F32, tag="o", name=f"o{j}")
                        layer_norm(8 + j, y_ps[:, 0:512], xown_sb[:, j, :],
                                   o_sb[:],
                                   bc.get("b2"), bc.get("g2"), bc.get("bt2"))
                        dma(out=out_d[128 * j:128 * j + 128, :], in_=o_sb[:])

    with tile.TileContext(nc) as tc:
        if reps == 1:
            body(tc)
        else:
            with tc.For_i(0, reps):
                body(tc)
    return nc


_NC_CACHE = {}


def _get_nc(flags):
    key = (tuple(sorted(flags.items())),
           os.environ.get("KERNEL_STOP_AFTER", ""),
           os.environ.get("KERNEL_REPS", "1"))
    if key not in _NC_CACHE:
        nc = bacc.Bacc("TRN2", target_bir_lowering=False, debug=False)
        _emit(nc, flags)
        nc.compile()
        _NC_CACHE[key] = nc
    return _NC_CACHE[key]


LAST_RESULTS = None


def make_in_maps(x, in_proj_w, in_proj_b, out_w, out_b, ln1_g, ln1_b, ln2_g,
                 ln2_b, ff_w1, ff_b1, ff_w2, ff_b2):
    x = np.asarray(x, dtype=np.float32)
    scale = np.float32(1.0 / np.sqrt(HD))

    wqkvT_f = np.ascontiguousarray(np.asarray(in_proj_w, np.float32).T)  # (D, 3D)
    wqkvT_f[:, :D] *= scale
    wqkvT = np.ascontiguousarray(wqkvT_f[:, :2 * D]).astype(ml_dtypes.bfloat16)
    wv8 = np.ascontiguousarray(wqkvT_f[:, 2 * D:]).astype(ml_dtypes.float8_e4m3fn)
    bqkv = np.asarray(in_proj_b, np.float32).copy()
    bqkv[:D] *= scale
    bqkv_pp = np.ascontiguousarray(bqkv.reshape(12, 128).T)
    woutT = np.ascontiguousarray(np.asarray(out_w, np.float32).T).astype(
        ml_dtypes.bfloat16)
    w1T = np.ascontiguousarray(np.asarray(ff_w1, np.float32).T).astype(
        ml_dtypes.float8_e4m3fn)
    b1_pp = np.ascontiguousarray(np.asarray(ff_b1, np.float32).reshape(16, 128).T)
    w2T = np.ascontiguousarray(np.asarray(ff_w2, np.float32).T).astype(
        ml_dtypes.bfloat16)

    assign = np.zeros((8, 4, 128), np.float32)
    for h in range(8):
        i = h // 2
        lo = 64 * (h % 2)
        assign[h, i, lo:lo + 64] = 1.0
    ident = np.eye(128, dtype=np.float32)

    bv = bqkv[2 * D:3 * D]
    vecs = np.stack([
        bv,
        np.asarray(out_b, np.float32),
        np.asarray(ff_b2, np.float32),
        np.asarray(ln1_g, np.float32),
        np.asarray(ln1_b, np.float32),
        np.asarray(ln2_g, np.float32),
        np.asarray(ln2_b, np.float32),
    ]).astype(np.float32)

    flags = {
        "bv": bool(np.any(bv != 0)),
        "bout": bool(np.any(vecs[1] != 0)),
        "b2": bool(np.any(vecs[2] != 0)),
        "g1": bool(np.any(vecs[3] != 1)),
        "bt1": bool(np.any(vecs[4] != 0)),
        "g2": bool(np.any(vecs[5] != 1)),
        "bt2": bool(np.any(vecs[6] != 0)),
    }

    in_maps = []
    for c in range(N_CORES):
        b, hh = c // 2, c % 2
        xb = x[b]
        xT = np.ascontiguousarray(xb.T) if hh == 0 else \
            np.ascontiguousarray(np.roll(xb.T, -SQ, axis=1))
        in_maps.append({
            "xT": xT.astype(ml_dtypes.bfloat16),
            "xT8": xT.astype(ml_dtypes.float8_e4m3fn), "wv8": wv8,
            "x_own": np.ascontiguousarray(xb[SQ * hh:SQ * (hh + 1)]),
            "wqkvT": wqkvT, "bqkv_pp": bqkv_pp, "woutT": woutT,
            "w1T": w1T, "b1_pp": b1_pp, "w2T": w2T,
            "assign": assign, "ident": ident, "vecs": vecs,
        })
    return in_maps, flags


def kernel(x, in_proj_w, in_proj_b, out_w, out_b, ln1_g, ln1_b, ln2_g, ln2_b,
           ff_w1, ff_b1, ff_w2, ff_b2):
    global LAST_RESULTS
    in_maps, flags = make_in_maps(
        x, in_proj_w, in_proj_b, out_w, out_b, ln1_g, ln1_b, ln2_g, ln2_b,
        ff_w1, ff_b1, ff_w2, ff_b2)
    nc = _get_nc(flags)
    res = run_bass_kernel_spmd(
        nc, in_maps, core_ids=list(range(N_CORES)),
        trace=bool(int(os.environ.get("BASS_KERNEL_TRACE", "0"))))
    LAST_RESULTS = res

    out = np.empty((B, S, D), np.float32)
    for c in range(N_CORES):
        b, hh = c // 2, c % 2
        out[b, SQ * hh:SQ * (hh + 1)] = res.results[c]["out"]
    return out



# revision 10
# speedup vs baseline: 1.1882x; 1.1882x over previous
"""Trainium2 Bass kernel for a dense transformer AttentionBlock.

Problem (fixed shapes): B=4, S=2048, D=512, H=8 heads (HD=64), FFN hidden 2048.
  qkv = x @ in_proj_w.T + b ; attn = softmax(q k^T / sqrt(64)) ; ctx = attn @ v
  x1 = LN(x + ctx @ out_w.T + out_b) ; out = LN(x1 + gelu(x1 @ w1.T + b1) @ w2.T + b2)

Sharding: 8 cores, zero collectives. Core c handles batch b=c//2, sequence half
h=c%2 (1024 query tokens). K/V are computed redundantly for the full 2048-token
sequence of the batch on both cores of a pair. One SPMD program for all cores:
for odd cores the host rolls x^T by -1024 columns so the core's own query
tokens always sit at columns [0,1024) (k-token order is irrelevant to softmax).

The kernel is scheduled around the softmax-exp wall: exp of the 8*2048*1024
score matrix runs only on the Activation engine (~128 x 1us instructions) and
is the binding resource of the attention phase, so everything else is arranged
to hide beneath it:
  - minimal prefix (Q head-pair 0, K half 0) starts the first exp ~12us in;
    V / remaining Q / K projections ride the PE queue interleaved under the
    head-pair-0 exp stream, with per-head-pair softmax normalization spaced
    into the NEXT pair's window so boundary scores never wait on a DVE chain.
  - an idle PE drops to 0.65GHz and needs ~3us to re-ramp to 2.4GHz, so the
    schedule never lets PE drain: throwaway warm-up matmuls cover the initial
    DMA wait.
Dtypes: weights/activations feeding matmuls are host-cast bf16 (halves DMA +
SBUF; PE rate is identical to f32r). Three paths run fp8e4m3 with DoubleRow
matmuls (2 contraction rows/cell/cycle): the V projection, attn @ V (exp
emits fp8 weights; the row-sum — via one-hot columns appended to V, landing
on psum rows 64:72 — is computed from the SAME quantized weights, so softmax
renormalization cancels the quantization almost exactly: measured +0e0 vs
bf16), and FFN1 (x2/w1, ~9e-3 rel err, the dominant error term). FFN2 stays
bf16: both FFN layers in fp8 measured 1.47e-2 — too close to the 2e-2 gate.
The residual/LayerNorm path stays f32.

Timing support: KERNEL_REPS>1 wraps the body in a tc.For_i HARDWARE loop so
program size stays constant while device work scales linearly; test.py
extracts true device time as the slope of wall time vs reps (per-call axon
dispatch overhead is ~2-3s, noisy, and grows with program size, so a
Python-unrolled rep loop measures the host, not the kernel).
"""

import os
import numpy as np
import ml_dtypes
from contextlib import ExitStack

import concourse.bass as bass
import concourse.mybir as mybir
import concourse.tile as tile
from concourse import bacc
from concourse.bass_utils import run_bass_kernel_spmd

F32 = mybir.dt.float32
F32R = mybir.dt.float32r
BF16 = mybir.dt.bfloat16
FP8 = mybir.dt.float8e4
PM = mybir.MatmulPerfMode
AF = mybir.ActivationFunctionType
OP = mybir.AluOpType

B, S, D, H = 4, 2048, 512, 8
HD = D // H          # 64
F = 4 * D            # 2048
SQ = S // 2          # 1024 own query tokens per core
EPS = 1e-5
N_CORES = 8

# vext: per head 72 columns = [v(64) | 8 filler]; ones at col 72*h + 64 + h
VW = 72
VEXT_W = H * VW      # 576


def _emit(nc, flags):
    """Emit the whole per-core program. flags: dict of bools for optional ops.
    KERNEL_STOP_AFTER in {qkv, attn, ln1, ffn1} truncates for cost analysis.
    KERNEL_REPS>1 wraps the body in a tc.For_i HARDWARE loop: the program size
    stays constant while device work scales linearly, so wall-time deltas
    between two reps values isolate true device execution time (per-call axon
    dispatch overhead is large, noisy, and scales with program size — a
    Python-unrolled rep loop measures that overhead, not the kernel)."""
    stop_after = os.environ.get("KERNEL_STOP_AFTER", "")
    reps = int(os.environ.get("KERNEL_REPS", "1"))
    # ---- DRAM parameters ----
    xT_d = nc.declare_dram_parameter("xT", [D, S], BF16, isOutput=False)
    xown_d = nc.declare_dram_parameter("x_own", [SQ, D], F32R, isOutput=False)
    wqkvT_d = nc.declare_dram_parameter("wqkvT", [D, 2 * D], BF16, isOutput=False)
    xT8_d = nc.declare_dram_parameter("xT8", [D, S], FP8, isOutput=False)
    wv8_d = nc.declare_dram_parameter("wv8", [D, D], FP8, isOutput=False)
    bqkv_d = nc.declare_dram_parameter("bqkv_pp", [128, 12], F32, isOutput=False)
    woutT_d = nc.declare_dram_parameter("woutT", [D, D], BF16, isOutput=False)
    w1T_d = nc.declare_dram_parameter("w1T", [D, F], FP8, isOutput=False)
    b1_d = nc.declare_dram_parameter("b1_pp", [128, 16], F32, isOutput=False)
    w2T_d = nc.declare_dram_parameter("w2T", [F, D], BF16, isOutput=False)
    assign_d = nc.declare_dram_parameter("assign", [8, 4, 128], F32R, isOutput=False)
    ident_d = nc.declare_dram_parameter("ident", [128, 128], F32R, isOutput=False)
    vecs_d = nc.declare_dram_parameter("vecs", [7, D], F32, isOutput=False)
    out_d = nc.declare_dram_parameter("out", [SQ, D], F32, isOutput=True)

    VEC_ROW = {"bv": 0, "bout": 1, "b2": 2, "g1": 3, "bt1": 4, "g2": 5, "bt2": 6}

    dma = nc.gpsimd.dma_start      # stores / misc (SWDGE on Pool)
    ldma = nc.sync.dma_start       # loads (HWDGE issued from idle SP engine)

    def bcast_row(pool, name, row):
        t = pool.tile([128, D], F32, tag=f"bc_{name}", name=f"bc_{name}")
        src = vecs_d[row]  # (D,)
        src_b = bass.AP(tensor=src.tensor, offset=src.offset,
                        ap=[[0, 128]] + list(src.ap))
        dma(out=t[:], in_=src_b)
        return t

    def body(tc):
        with ExitStack() as es:
            persist = es.enter_context(tc.tile_pool(name="persist", bufs=1))
            work = es.enter_context(tc.tile_pool(name="work", bufs=4))
            psum = es.enter_context(tc.tile_pool(name="psum", bufs=2, space="PSUM"))
            xo = es.enter_context(tc.tile_pool(name="xo", bufs=1))
            shr = es.enter_context(tc.tile_pool(name="shr", bufs=1))
            w1p = es.enter_context(tc.tile_pool(name="w1p", bufs=1))

            def ps_big(nm):
                # 4-bank ctx accumulator (one buffer)
                return psum.tile([128, 2048], F32, tag="c", name=nm, bufs=1)

            def ps_tile(nm, dt=F32):
                # 2-bank double-buffered working psum
                return psum.tile([128, 1024], dt, tag="s", name=nm, bufs=2)

            # ---- tiny persistent tensors (cheap DMAs; big loads below) ----
            bqkv_sb = persist.tile([128, 12], F32, name="bqkv_sb")
            ldma(out=bqkv_sb[:], in_=bqkv_d[:])
            b1_sb = persist.tile([128, 16], F32, name="b1_sb")
            ldma(out=b1_sb[:], in_=b1_d[:])
            eps_sb = persist.tile([128, 1], F32, name="eps_sb")
            nc.vector.memset(eps_sb[:], EPS)
            bc = {}
            for nm in ("bv", "bout", "b2", "g1", "bt1", "g2", "bt2"):
                if flags[nm]:
                    bc[nm] = bcast_row(persist, nm, VEC_ROW[nm])
            # tiles whose loads are deferred off the critical DMA path
            woutT_sb = persist.tile([128, 4, D], BF16, name="woutT_sb")
            assign_sb = persist.tile([128, 4, 128], F32R, name="assign_sb")
            ident_sb = persist.tile([128, 128], F32R, name="ident_sb")
            xown_sb = xo.tile([128, 8, D], F32R, name="xown_sb")
            # shared feature-major activation tile: ctx^T, later x2^T
            actT_sb = shr.tile([128, 4, SQ], BF16, name="actT_sb")
            # FFN w1, prefetched during attention
            w1T_sb = w1p.tile([128, 4, F], FP8, name="w1T_sb")
            actT8_sb = w1p.tile([128, 4, SQ], FP8, name="actT8_sb")

            with tc.tile_pool(name="qk", bufs=1) as qk:
                qT_sb = qk.tile([128, 4, SQ], BF16, name="qT_sb")
                kT_sb = qk.tile([128, 4, S], BF16, name="kT_sb")
                vext_sb = qk.tile([128, 16, VEXT_W], FP8, name="vext_sb")
                # vext filler: zero cols 64:72 per head, then 1.0 at col 64+h
                # (row-sum one-hot). Pool memsets, no DMA traffic.
                vfill = vext_sb[:].rearrange("p t (h w) -> p t h w", w=VW)
                nc.gpsimd.memset(vfill[:, :, :, HD:VW], 0.0)
                for h in range(H):
                    nc.gpsimd.memset(vfill[:, :, h, HD + h:HD + h + 1], 1.0)

                rsum_sb = qk.tile([128, SQ], F32R, name="rsum_sb")  # 64:72
                with tc.tile_pool(name="pp", bufs=4) as pp, \
                     tc.tile_pool(name="pha", bufs=1) as pha:
                    xT_sb = pha.tile([128, 4, S], BF16, name="xT_sb")
                    wqkvT_sb = pha.tile([128, 4, 2 * D], BF16, name="wqkvT_sb")
                    xT8_sb = pha.tile([128, 4, S], FP8, name="xT8_sb")
                    wv8_sb = pha.tile([128, 4, D], FP8, name="wv8_sb")

                    # ---- load order = need order: Q needs xT cols 0:1024 +
                    # wqkv cols 0:512; K0 next; V cols next; rest deferred.
                    xT_src = xT_d.ap().rearrange("(c p) t -> p c t", p=128)
                    for tq in range(2):
                        ldma(out=xT_sb[:, :, 512 * tq:512 * tq + 512],
                             in_=xT_src[:, :, 512 * tq:512 * tq + 512])
                    for c in range(4):   # Q weight cols (first exp needs them)
                        ldma(out=wqkvT_sb[:, c, 0:512],
                             in_=wqkvT_d[128 * c:128 * c + 128, 0:512])
                    for c in range(4):   # K weight cols
                        ldma(out=wqkvT_sb[:, c, 512:1024],
                             in_=wqkvT_d[128 * c:128 * c + 128, 512:1024])
                    xT8_src = xT8_d.ap().rearrange("(c p) t -> p c t", p=128)
                    ldma(out=wv8_sb[:],
                         in_=wv8_d.ap().rearrange("(c p) m -> p c m", p=128))
                    for tq8 in range(2):
                        ldma(out=xT8_sb[:, :, 1024 * tq8:1024 * tq8 + 1024],
                             in_=xT8_src[:, :, 1024 * tq8:1024 * tq8 + 1024])
                    for tq in range(2, 4):
                        ldma(out=xT_sb[:, :, 512 * tq:512 * tq + 512],
                             in_=xT_src[:, :, 512 * tq:512 * tq + 512])
                    # off-critical-path loads (after the attention-feeding ones)
                    ldma(out=xown_sb[:],
                         in_=xown_d.ap().rearrange("(j p) d -> p j d", p=128))
                    ldma(out=woutT_sb[:],
                         in_=woutT_d.ap().rearrange("(c p) m -> p c m", p=128))
                    ldma(out=assign_sb[64:72, :, :], in_=assign_d[:])
                    ldma(out=ident_sb[:], in_=ident_d[:])
                    ldma(out=w1T_sb[:],
                         in_=w1T_d.ap().rearrange("(c p) m -> p c m", p=128))
                    if stop_after == "dma":
                        return

                    def emit_q(mi):
                        q_ps = ps_tile(f"q_ps{mi}")
                        for tj in range(2):
                            for c in range(4):
                                nc.tensor.matmul(
                                    q_ps[:, 512 * tj:512 * tj + 512],
                                    wqkvT_sb[:, c, 128 * mi:128 * mi + 128],
                                    xT_sb[:, c, 512 * tj:512 * tj + 512],
                                    start=(c == 0), stop=(c == 3))
                        nc.vector.tensor_scalar(
                            qT_sb[:, mi, :], q_ps[:],
                            bqkv_sb[:, mi:mi + 1], None, OP.add)

                    def emit_k_half(mi, kh):
                        # K^T for head-pair mi, token half kh; evac on DVE
                        # (keeps ACT free for softmax exp)
                        k_ps = ps_tile(f"k_ps{mi}_{kh}")
                        for tj in range(2):
                            for c in range(4):
                                nc.tensor.matmul(
                                    k_ps[:, 512 * tj:512 * tj + 512],
                                    wqkvT_sb[:, c,
                                             512 + 128 * mi:512 + 128 * mi + 128],
                                    xT_sb[:, c, 1024 * kh + 512 * tj:
                                          1024 * kh + 512 * tj + 512],
                                    start=(c == 0), stop=(c == 3))
                        nc.vector.tensor_scalar(
                            kT_sb[:, mi, 1024 * kh:1024 * kh + 1024],
                            k_ps[:], bqkv_sb[:, 4 + mi:5 + mi], None,
                            OP.add)

                    def emit_k(mi):
                        emit_k_half(mi, 0)
                        emit_k_half(mi, 1)

                    def emit_v(ti):
                        # fp8 DoubleRow: two 128-feature contraction chunks
                        # per instruction (V tolerates fp8 inputs well; the
                        # attention average washes element noise out)
                        v_ps = ps_tile(f"v_ps{ti}")
                        for cp in range(2):
                            nc.tensor.matmul(
                                v_ps[:, 0:512],
                                xT8_sb[:, 2 * cp:2 * cp + 2,
                                       128 * ti:128 * ti + 128],
                                wv8_sb[:, 2 * cp:2 * cp + 2, :],
                                start=(cp == 0), stop=(cp == 1),
                                perf_mode=PM.DoubleRow)
                        v_dst = vext_sb[:, ti, :].rearrange(
                            "p (h e) -> p h e", e=VW)[:, :, 0:HD]
                        v_src = v_ps[:, 0:512].rearrange("p (h e) -> p h e", e=HD)
                        if flags["bv"]:
                            nc.vector.tensor_tensor(
                                v_dst, v_src,
                                bc["bv"][:].rearrange("p (h e) -> p h e", e=HD),
                                OP.add)
                        else:
                            nc.vector.tensor_copy(v_dst, v_src)

                    def attn_step(hp, ktp, hh, c_ps, ppool):
                        # scores -> exp -> ctx for one (head-pair, k-tile
                        # PAIR, hh). exp emits fp8 attention weights for the
                        # two k-tiles side by side; ctx then contracts both
                        # in one DoubleRow matmul per 512-token column chunk
                        # (2 fp8 contraction rows per PE cell per cycle).
                        # Softmax renormalizes by the sum of the same fp8
                        # weights (one-hot V columns), so weight quantization
                        # largely cancels.
                        h = 2 * hp + hh
                        p_sb = ppool.tile([128, 2, 1024], FP8, tag="p",
                                          name=f"p{hp}_{ktp}_{hh}")
                        for ki in range(2):
                            kt = 2 * ktp + ki
                            s_ps = ps_tile(f"s_ps{hp}_{ktp}_{hh}_{ki}")
                            lhsT = kT_sb[64 * hh:64 * hh + 64, hp,
                                         128 * kt:128 * kt + 128]
                            for tj in range(2):
                                nc.tensor.matmul(
                                    s_ps[:, 512 * tj:512 * tj + 512],
                                    lhsT,
                                    qT_sb[64 * hh:64 * hh + 64, hp,
                                          512 * tj:512 * tj + 512],
                                    start=True, stop=True)
                            nc.scalar.activation(out=p_sb[:, ki, :],
                                                 in_=s_ps[:], func=AF.Exp)
                        lhsT = vext_sb[:].rearrange(
                            "p t w -> p t w")[:, 2 * ktp:2 * ktp + 2,
                                              VW * h:VW * h + VW]
                        for tj in range(2):
                            nc.tensor.matmul(
                                c_ps[0:VW, 1024 * hh + 512 * tj:
                                     1024 * hh + 512 * tj + 512],
                                lhsT,
                                p_sb[:, :, 512 * tj:512 * tj + 512],
                                start=(ktp == 0), stop=(ktp == 7),
                                perf_mode=PM.DoubleRow)

                    def evac_attn(hp, c_ps):
                        # ctx^T rows 0:64 -> actT. Head 2hp+hh's row-sum sits
                        # on psum row 64+2hp+hh of column half hh (one-hot V
                        # column; other heads' rows are zero there, so the
                        # aligned 8-row block accumulates cleanly).
                        for hh in range(2):
                            nc.vector.tensor_copy(
                                actT_sb[64 * hh:64 * hh + 64, hp, :],
                                c_ps[0:64, 1024 * hh:1024 * hh + SQ])
                            if hp == 0 and hh == 0:
                                nc.vector.tensor_copy(
                                    rsum_sb[64:72, :], c_ps[64:72, 0:SQ])
                            else:
                                nc.vector.tensor_tensor(
                                    rsum_sb[64:72, :], rsum_sb[64:72, :],
                                    c_ps[64:72, 1024 * hh:1024 * hh + SQ],
                                    OP.add)

                    # PE p-state warmup: the array clocks 0.65->2.4GHz
                    # only after ~3us of continuous work; run throwaway
                    # matmuls on a zeroed scratch tile while the first xT/w
                    # DMAs are in flight so Q/K/scores start at full clock.
                    scr = pha.tile([128, 512], BF16, name="scr_sb")
                    nc.gpsimd.memset(scr[:], 0.0)
                    for wu in range(8):
                        w_ps = ps_tile(f"wu{wu}")
                        nc.tensor.matmul(w_ps[:, 0:512], scr[:, 0:128],
                                         scr[:], start=True, stop=True)
                    # Minimal prefix before the first softmax exp: Q(mi0)
                    # and K(0, token-half 0) — everything else (Q1-3, K0h1,
                    # V, K1) rides the PE queue interleaved under hp0's exp
                    # wall. hp0's first 8 k-tiles live in token half 0.
                    emit_q(0)
                    emit_k_half(0, 0)
                    if stop_after == "qkv":
                        for mi in range(1, 4):
                            emit_q(mi)
                        emit_k_half(0, 1)
                        for ti in range(16):
                            emit_v(ti)
                        for mi in range(1, 4):
                            emit_k(mi)
                        return

                    def norm_hp(hp):
                        # per-head-pair softmax denominator. rsum rows for
                        # this hp are final after evac_attn(hp) (later adds
                        # only contribute zeros); stale rows of other pairs
                        # are masked by zeros in assign. The broadcast matmul
                        # runs on the RAW sums; the reciprocal is taken on
                        # the partition-0-aligned broadcast result (single
                        # partitions 65.. are not engine-addressable).
                        n_ps = ps_tile(f"n_ps{hp}")
                        for tj in range(2):
                            nc.tensor.matmul(
                                n_ps[:, 512 * tj:512 * tj + 512],
                                assign_sb[64:72, hp, :],
                                rsum_sb[64:72, 512 * tj:512 * tj + 512],
                                start=True, stop=True)
                        nrec = work.tile([128, SQ], F32, tag="nr",
                                         name=f"nrec{hp}")
                        nc.vector.reciprocal(nrec[:], n_ps[:, 0:SQ])
                        with nc.allow_low_precision(
                                reason="attention weights tolerate bf16"):
                            nc.vector.tensor_tensor(
                                actT_sb[:, hp, :], actT_sb[:, hp, :],
                                nrec[:], OP.mult)

                    c_ps0 = ps_big("c_ps0")
                    for ktp in range(8):
                        emit_v(2 * ktp)
                        emit_v(2 * ktp + 1)
                        attn_step(0, ktp, 0, c_ps0, pp)
                        attn_step(0, ktp, 1, c_ps0, pp)
                        if ktp < 3:
                            emit_q(ktp + 1)
                        elif ktp == 3:
                            emit_k_half(0, 1)
                        elif ktp in (4, 6):
                            emit_k_half(1, (ktp - 4) // 2)
                    evac_attn(0, c_ps0)
                    for hp in range(1, 4):
                        c_ps = ps_big(f"c_ps{hp}")
                        for ktp in range(8):
                            attn_step(hp, ktp, 0, c_ps, pp)
                            attn_step(hp, ktp, 1, c_ps, pp)
                            if hp < 3 and ktp in (2, 5):
                                emit_k_half(hp + 1, (ktp - 2) // 3)
                            elif ktp == 1:
                                norm_hp(hp - 1)
                        evac_attn(hp, c_ps)
                    norm_hp(3)
                    # keep the PE array clocked through the evac/normalize
                    # DVE chain (an idle PE drops to 0.65GHz and would crawl
                    # through the first attn-out matmuls while re-ramping)
                    for wu in range(8):
                        w_ps = ps_tile(f"wt{wu}")
                        nc.tensor.matmul(w_ps[:, 0:512], scr[:, 0:128],
                                         scr[:], start=True, stop=True)



            if stop_after == "attn":
                return
            # ---- attn_out (natural) + LN1 -> x2 (in place over x_own) ----
            def layer_norm(j, acc_ps, resid_ap, out_ap, pre_b, g, bt):
                z = work.tile([128, D], F32, tag="z", name=f"z{j}")
                nc.vector.tensor_tensor(z[:], resid_ap, acc_ps, OP.add)
                if pre_b is not None:
                    nc.vector.tensor_tensor(z[:], z[:], pre_b[:], OP.add)
                st = work.tile([128, 6], F32, tag="st", name=f"st{j}")
                nc.vector.bn_stats(out=st[:], in_=z[:])
                mv = work.tile([128, 2], F32, tag="mv", name=f"mv{j}")
                nc.vector.bn_aggr(out=mv[:], in_=st[:])
                sd = work.tile([128, 1], F32, tag="sd", name=f"sd{j}")
                nc.scalar.activation(out=sd[:], in_=mv[:, 1:2], func=AF.Sqrt,
                                     bias=eps_sb[:], scale=1.0)
                nc.vector.reciprocal(sd[:], sd[:])
                nc.vector.tensor_scalar(out_ap, z[:], mv[:, 0:1], sd[:],
                                        OP.subtract, OP.mult)
                if g is not None:
                    nc.vector.tensor_tensor(out_ap, out_ap, g[:], OP.mult)
                if bt is not None:
                    nc.vector.tensor_tensor(out_ap, out_ap, bt[:], OP.add)

            if True:
                for j in range(8):
                    a_ps = ps_tile(f"a_ps{j}")
                    for c in range(4):
                        nc.tensor.matmul(a_ps[:, 0:512],
                                         actT_sb[:, c, 128 * j:128 * j + 128],
                                         woutT_sb[:, c, :],
                                         start=(c == 0), stop=(c == 3))
                    layer_norm(j, a_ps[:, 0:512], xown_sb[:, j, :],
                               xown_sb[:, j, :],
                               bc.get("bout"), bc.get("g1"), bc.get("bt1"))

                if stop_after == "ln1":
                    return
                # ============ phase 3: FFN + LN2 ============
                # x2^T via PE transposes (into actT, reusing the ctx^T tile)
                for i in range(4):
                    t_ps = ps_tile(f"t_ps{i}", F32R)
                    for j in range(8):
                        nc.tensor.transpose(t_ps[:, 128 * j:128 * j + 128],
                                            xown_sb[:, j, 128 * i:128 * i + 128],
                                            ident_sb[:])
                    with nc.allow_low_precision(
                            reason="x2 feeds fp8 DoubleRow FFN matmuls"):
                        nc.vector.tensor_copy(actT8_sb[:, i, :], t_ps[:])

                with tc.tile_pool(name="hp_", bufs=1) as hpool, \
                     tc.tile_pool(name="w2p", bufs=1) as w2p:
                    w2T_sb = w2p.tile([128, 16, D], BF16, name="w2T_sb")
                    for c in range(0, 16, 4):
                        dma(out=w2T_sb[:, c:c + 4, :],
                            in_=w2T_d.ap().rearrange("(c p) m -> p c m",
                                                     p=128)[:, c:c + 4, :])
                    hT_sb = hpool.tile([128, 16, SQ], BF16, name="hT_sb")

                    def ffn1_m(m):
                        f_ps = ps_tile(f"f_ps{m}")
                        for tj in range(2):
                            for cp in range(2):
                                nc.tensor.matmul(
                                    f_ps[:, 512 * tj:512 * tj + 512],
                                    w1T_sb[:, 2 * cp:2 * cp + 2,
                                           128 * m:128 * m + 128],
                                    actT8_sb[:, 2 * cp:2 * cp + 2,
                                             512 * tj:512 * tj + 512],
                                    start=(cp == 0), stop=(cp == 1),
                                    perf_mode=PM.DoubleRow)
                        nc.scalar.activation(out=hT_sb[:, m, :], in_=f_ps[:],
                                             func=AF.Gelu,
                                             bias=b1_sb[:, m:m + 1], scale=1.0)

                    def ffn2_fc(y_ps, wave, fc):
                        for jj in range(4):
                            j = 4 * wave + jj
                            nc.tensor.matmul(
                                y_ps[:, 512 * jj:512 * jj + 512],
                                hT_sb[:, fc, 128 * j:128 * j + 128],
                                w2T_sb[:, fc, :],
                                start=(fc == 0), stop=(fc == 15))

                    def ln2_wave(y_ps, wave):
                        for jj in range(4):
                            j = 4 * wave + jj
                            o_sb = work.tile([128, D], F32, tag="o",
                                             name=f"o{j}")
                            layer_norm(8 + j, y_ps[:, 512 * jj:512 * jj + 512],
                                       xown_sb[:, j, :], o_sb[:],
                                       bc.get("b2"), bc.get("g2"),
                                       bc.get("bt2"))
                            dma(out=out_d[128 * j:128 * j + 128, :],
                                in_=o_sb[:])

                    for m in range(16):
                        ffn1_m(m)
                    if stop_after == "ffn1":
                        return
                    # plain per-token-block FFN2: each block's 16-chunk psum
                    # accumulation chases the gelu stream naturally (fc<m
                    # chunks run while later gelus are still in flight)
                    for j in range(8):
                        y_ps = ps_tile(f"y_ps{j}")
                        for fc in range(16):
                            nc.tensor.matmul(y_ps[:, 0:512],
                                             hT_sb[:, fc, 128 * j:128 * j + 128],
                                             w2T_sb[:, fc, :],
                                             start=(fc == 0), stop=(fc == 15))
                        o_sb = work.tile([128, D], F32, tag="o", name=f"o{j}")
                        layer_norm(8 + j, y_ps[:, 0:512], xown_sb[:, j, :],
                                   o_sb[:],
                                   bc.get("b2"), bc.get("g2"), bc.get("bt2"))
                        dma(out=out_d[128 * j:128 * j + 128, :], in_=o_sb[:])

    with tile.TileContext(nc) as tc:
        if reps == 1:
            body(tc)
        else:
            with tc.For_i(0, reps):
                body(tc)
    return nc


_NC_CACHE = {}


def _get_nc(flags):
    key = (tuple(sorted(flags.items())),
           os.environ.get("KERNEL_STOP_AFTER", ""),
           os.environ.get("KERNEL_REPS", "1"))
    if key not in _NC_CACHE:
        nc = bacc.Bacc("TRN2", target_bir_lowering=False, debug=False)
        _emit(nc, flags)
        nc.compile()
        _NC_CACHE[key] = nc
    return _NC_CACHE[key]


LAST_RESULTS = None


def make_in_maps(x, in_proj_w, in_proj_b, out_w, out_b, ln1_g, ln1_b, ln2_g,
                 ln2_b, ff_w1, ff_b1, ff_w2, ff_b2):
    x = np.asarray(x, dtype=np.float32)
    scale = np.float32(1.0 / np.sqrt(HD))

    wqkvT_f = np.ascontiguousarray(np.asarray(in_proj_w, np.float32).T)  # (D, 3D)
    wqkvT_f[:, :D] *= scale
    wqkvT = np.ascontiguousarray(wqkvT_f[:, :2 * D]).astype(ml_dtypes.bfloat16)
    wv8 = np.ascontiguousarray(wqkvT_f[:, 2 * D:]).astype(ml_dtypes.float8_e4m3fn)
    bqkv = np.asarray(in_proj_b, np.float32).copy()
    bqkv[:D] *= scale
    bqkv_pp = np.ascontiguousarray(bqkv.reshape(12, 128).T)
    woutT = np.ascontiguousarray(np.asarray(out_w, np.float32).T).astype(
        ml_dtypes.bfloat16)
    w1T = np.ascontiguousarray(np.asarray(ff_w1, np.float32).T).astype(
        ml_dtypes.float8_e4m3fn)
    b1_pp = np.ascontiguousarray(np.asarray(ff_b1, np.float32).reshape(16, 128).T)
    w2T = np.ascontiguousarray(np.asarray(ff_w2, np.float32).T).astype(
        ml_dtypes.bfloat16)

    assign = np.zeros((8, 4, 128), np.float32)
    for h in range(8):
        i = h // 2
        lo = 64 * (h % 2)
        assign[h, i, lo:lo + 64] = 1.0
    ident = np.eye(128, dtype=np.float32)

    bv = bqkv[2 * D:3 * D]
    vecs = np.stack([
        bv,
        np.asarray(out_b, np.float32),
        np.asarray(ff_b2, np.float32),
        np.asarray(ln1_g, np.float32),
        np.asarray(ln1_b, np.float32),
        np.asarray(ln2_g, np.float32),
        np.asarray(ln2_b, np.float32),
    ]).astype(np.float32)

    flags = {
        "bv": bool(np.any(bv != 0)),
        "bout": bool(np.any(vecs[1] != 0)),
        "b2": bool(np.any(vecs[2] != 0)),
        "g1": bool(np.any(vecs[3] != 1)),
        "bt1": bool(np.any(vecs[4] != 0)),
        "g2": bool(np.any(vecs[5] != 1)),
        "bt2": bool(np.any(vecs[6] != 0)),
    }

    in_maps = []
    for c in range(N_CORES):
        b, hh = c // 2, c % 2
        xb = x[b]
        xT = np.ascontiguousarray(xb.T) if hh == 0 else \
            np.ascontiguousarray(np.roll(xb.T, -SQ, axis=1))
        in_maps.append({
            "xT": xT.astype(ml_dtypes.bfloat16),
            "xT8": xT.astype(ml_dtypes.float8_e4m3fn), "wv8": wv8,
            "x_own": np.ascontiguousarray(xb[SQ * hh:SQ * (hh + 1)]),
            "wqkvT": wqkvT, "bqkv_pp": bqkv_pp, "woutT": woutT,
            "w1T": w1T, "b1_pp": b1_pp, "w2T": w2T,
            "assign": assign, "ident": ident, "vecs": vecs,
        })
    return in_maps, flags


def kernel(x, in_proj_w, in_proj_b, out_w, out_b, ln1_g, ln1_b, ln2_g, ln2_b,
           ff_w1, ff_b1, ff_w2, ff_b2):
    global LAST_RESULTS
    in_maps, flags = make_in_maps(
        x, in_proj_w, in_proj_b, out_w, out_b, ln1_g, ln1_b, ln2_g, ln2_b,
        ff_w1, ff_b1, ff_w2, ff_b2)
    nc = _get_nc(flags)
    res = run_bass_kernel_spmd(
        nc, in_maps, core_ids=list(range(N_CORES)),
        trace=bool(int(os.environ.get("BASS_KERNEL_TRACE", "0"))))
    LAST_RESULTS = res

    out = np.empty((B, S, D), np.float32)
    for c in range(N_CORES):
        b, hh = c // 2, c % 2
        out[b, SQ * hh:SQ * (hh + 1)] = res.results[c]["out"]
    return out



# revision 11
# speedup vs baseline: 1.1959x; 1.0065x over previous
"""Trainium2 Bass kernel for a dense transformer AttentionBlock.

Problem (fixed shapes): B=4, S=2048, D=512, H=8 heads (HD=64), FFN hidden 2048.
  qkv = x @ in_proj_w.T + b ; attn = softmax(q k^T / sqrt(64)) ; ctx = attn @ v
  x1 = LN(x + ctx @ out_w.T + out_b) ; out = LN(x1 + gelu(x1 @ w1.T + b1) @ w2.T + b2)

Sharding: 8 cores, zero collectives. Core c handles batch b=c//2, sequence half
h=c%2 (1024 query tokens). K/V are computed redundantly for the full 2048-token
sequence of the batch on both cores of a pair. One SPMD program for all cores:
for odd cores the host rolls x^T by -1024 columns so the core's own query
tokens always sit at columns [0,1024) (k-token order is irrelevant to softmax).

The kernel is scheduled around the softmax-exp wall: exp of the 8*2048*1024
score matrix runs only on the Activation engine (~128 x 1us instructions) and
is the binding resource of the attention phase, so everything else is arranged
to hide beneath it:
  - minimal prefix (Q head-pair 0, K half 0) starts the first exp ~12us in;
    V / remaining Q / K projections ride the PE queue interleaved under the
    head-pair-0 exp stream, with per-head-pair softmax normalization spaced
    into the NEXT pair's window so boundary scores never wait on a DVE chain.
  - an idle PE drops to 0.65GHz and needs ~3us to re-ramp to 2.4GHz, so the
    schedule never lets PE drain: throwaway warm-up matmuls cover the initial
    DMA wait.
Dtypes: weights/activations feeding matmuls are host-cast bf16 (halves DMA +
SBUF; PE rate is identical to f32r). Three paths run fp8e4m3 with DoubleRow
matmuls (2 contraction rows/cell/cycle): the V projection, attn @ V (exp
emits fp8 weights; the row-sum — via one-hot columns appended to V, landing
on psum rows 64:72 — is computed from the SAME quantized weights, so softmax
renormalization cancels the quantization almost exactly: measured +0e0 vs
bf16), and FFN1 (x2/w1, ~9e-3 rel err, the dominant error term). FFN2 stays
bf16: both FFN layers in fp8 measured 1.47e-2 — too close to the 2e-2 gate.
The residual/LayerNorm path stays f32.

Timing support: KERNEL_REPS>1 wraps the body in a tc.For_i HARDWARE loop so
program size stays constant while device work scales linearly; test.py
extracts true device time as the slope of wall time vs reps (per-call axon
dispatch overhead is ~2-3s, noisy, and grows with program size, so a
Python-unrolled rep loop measures the host, not the kernel).
"""

import os
import numpy as np
import ml_dtypes
from contextlib import ExitStack

import concourse.bass as bass
import concourse.mybir as mybir
import concourse.tile as tile
from concourse import bacc
from concourse.bass_utils import run_bass_kernel_spmd

F32 = mybir.dt.float32
F32R = mybir.dt.float32r
BF16 = mybir.dt.bfloat16
FP8 = mybir.dt.float8e4
PM = mybir.MatmulPerfMode
AF = mybir.ActivationFunctionType
OP = mybir.AluOpType

B, S, D, H = 4, 2048, 512, 8
HD = D // H          # 64
F = 4 * D            # 2048
SQ = S // 2          # 1024 own query tokens per core
EPS = 1e-5
N_CORES = 8

# vext: per head 72 columns = [v(64) | 8 filler]; ones at col 72*h + 64 + h
VW = 72
VEXT_W = H * VW      # 576


def _emit(nc, flags):
    """Emit the whole per-core program. flags: dict of bools for optional ops.
    KERNEL_STOP_AFTER in {qkv, attn, ln1, ffn1} truncates for cost analysis.
    KERNEL_REPS>1 wraps the body in a tc.For_i HARDWARE loop: the program size
    stays constant while device work scales linearly, so wall-time deltas
    between two reps values isolate true device execution time (per-call axon
    dispatch overhead is large, noisy, and scales with program size — a
    Python-unrolled rep loop measures that overhead, not the kernel)."""
    stop_after = os.environ.get("KERNEL_STOP_AFTER", "")
    reps = int(os.environ.get("KERNEL_REPS", "1"))
    # ---- DRAM parameters ----
    xT_d = nc.declare_dram_parameter("xT", [D, S], BF16, isOutput=False)
    xown_d = nc.declare_dram_parameter("x_own", [SQ, D], F32R, isOutput=False)
    wqkvT_d = nc.declare_dram_parameter("wqkvT", [D, 2 * D], BF16, isOutput=False)
    xT8_d = nc.declare_dram_parameter("xT8", [D, S], FP8, isOutput=False)
    wv8_d = nc.declare_dram_parameter("wv8", [D, D], FP8, isOutput=False)
    bqkv_d = nc.declare_dram_parameter("bqkv_pp", [128, 12], F32, isOutput=False)
    woutT_d = nc.declare_dram_parameter("woutT", [D, D], BF16, isOutput=False)
    w1T_d = nc.declare_dram_parameter("w1T", [D, F], FP8, isOutput=False)
    b1_d = nc.declare_dram_parameter("b1_pp", [128, 16], F32, isOutput=False)
    w2T_d = nc.declare_dram_parameter("w2T", [F, D], BF16, isOutput=False)
    assign_d = nc.declare_dram_parameter("assign", [8, 4, 128], F32R, isOutput=False)
    ident_d = nc.declare_dram_parameter("ident", [128, 128], F32R, isOutput=False)
    vecs_d = nc.declare_dram_parameter("vecs", [7, D], F32, isOutput=False)
    out_d = nc.declare_dram_parameter("out", [SQ, D], F32, isOutput=True)

    VEC_ROW = {"bv": 0, "bout": 1, "b2": 2, "g1": 3, "bt1": 4, "g2": 5, "bt2": 6}

    dma = nc.gpsimd.dma_start      # stores / misc (SWDGE on Pool)
    ldma = nc.sync.dma_start       # loads (HWDGE issued from idle SP engine)

    def bcast_row(pool, name, row):
        t = pool.tile([128, D], F32, tag=f"bc_{name}", name=f"bc_{name}")
        src = vecs_d[row]  # (D,)
        src_b = bass.AP(tensor=src.tensor, offset=src.offset,
                        ap=[[0, 128]] + list(src.ap))
        dma(out=t[:], in_=src_b)
        return t

    def body(tc):
        with ExitStack() as es:
            persist = es.enter_context(tc.tile_pool(name="persist", bufs=1))
            work = es.enter_context(tc.tile_pool(name="work", bufs=4))
            psum = es.enter_context(tc.tile_pool(name="psum", bufs=2, space="PSUM"))
            xo = es.enter_context(tc.tile_pool(name="xo", bufs=1))
            shr = es.enter_context(tc.tile_pool(name="shr", bufs=1))
            w1p = es.enter_context(tc.tile_pool(name="w1p", bufs=1))

            def ps_big(nm):
                # 4-bank ctx accumulator (one buffer)
                return psum.tile([128, 2048], F32, tag="c", name=nm, bufs=1)

            def ps_tile(nm, dt=F32):
                # 2-bank double-buffered working psum
                return psum.tile([128, 1024], dt, tag="s", name=nm, bufs=2)

            # ---- tiny persistent tensors (cheap DMAs; big loads below) ----
            bqkv_sb = persist.tile([128, 12], F32, name="bqkv_sb")
            ldma(out=bqkv_sb[:], in_=bqkv_d[:])
            b1_sb = persist.tile([128, 16], F32, name="b1_sb")
            ldma(out=b1_sb[:], in_=b1_d[:])
            eps_sb = persist.tile([128, 1], F32, name="eps_sb")
            nc.vector.memset(eps_sb[:], EPS)
            bc = {}
            for nm in ("bv", "bout", "b2", "g1", "bt1", "g2", "bt2"):
                if flags[nm]:
                    bc[nm] = bcast_row(persist, nm, VEC_ROW[nm])
            # tiles whose loads are deferred off the critical DMA path
            woutT_sb = persist.tile([128, 4, D], BF16, name="woutT_sb")
            assign_sb = persist.tile([128, 4, 128], F32R, name="assign_sb")
            ident_sb = persist.tile([128, 128], F32R, name="ident_sb")
            xown_sb = xo.tile([128, 8, D], F32R, name="xown_sb")
            # shared feature-major activation tile: ctx^T, later x2^T
            actT_sb = shr.tile([128, 4, SQ], BF16, name="actT_sb")
            # FFN w1, prefetched during attention
            w1T_sb = w1p.tile([128, 4, F], FP8, name="w1T_sb")
            actT8_sb = w1p.tile([128, 4, SQ], FP8, name="actT8_sb")

            with tc.tile_pool(name="qk", bufs=1) as qk:
                qT_sb = qk.tile([128, 4, SQ], BF16, name="qT_sb")
                kT_sb = qk.tile([128, 4, S], BF16, name="kT_sb")
                vext_sb = qk.tile([128, 16, VEXT_W], FP8, name="vext_sb")
                # vext filler: zero cols 64:72 per head, then 1.0 at col 64+h
                # (row-sum one-hot). Pool memsets, no DMA traffic.
                vfill = vext_sb[:].rearrange("p t (h w) -> p t h w", w=VW)
                nc.gpsimd.memset(vfill[:, :, :, HD:VW], 0.0)
                for h in range(H):
                    nc.gpsimd.memset(vfill[:, :, h, HD + h:HD + h + 1], 1.0)

                rsum_sb = qk.tile([128, SQ], F32R, name="rsum_sb")  # 64:72
                with tc.tile_pool(name="pp", bufs=4) as pp, \
                     tc.tile_pool(name="pha", bufs=1) as pha:
                    xT_sb = pha.tile([128, 4, S], BF16, name="xT_sb")
                    wqkvT_sb = pha.tile([128, 4, 2 * D], BF16, name="wqkvT_sb")
                    xT8_sb = pha.tile([128, 4, S], FP8, name="xT8_sb")
                    wv8_sb = pha.tile([128, 4, D], FP8, name="wv8_sb")

                    # ---- load order = need order: Q needs xT cols 0:1024 +
                    # wqkv cols 0:512; K0 next; V cols next; rest deferred.
                    xT_src = xT_d.ap().rearrange("(c p) t -> p c t", p=128)
                    for tq in range(2):
                        ldma(out=xT_sb[:, :, 512 * tq:512 * tq + 512],
                             in_=xT_src[:, :, 512 * tq:512 * tq + 512])
                    for c in range(4):   # Q weight cols (first exp needs them)
                        ldma(out=wqkvT_sb[:, c, 0:512],
                             in_=wqkvT_d[128 * c:128 * c + 128, 0:512])
                    for c in range(4):   # K weight cols
                        ldma(out=wqkvT_sb[:, c, 512:1024],
                             in_=wqkvT_d[128 * c:128 * c + 128, 512:1024])
                    xT8_src = xT8_d.ap().rearrange("(c p) t -> p c t", p=128)
                    ldma(out=wv8_sb[:],
                         in_=wv8_d.ap().rearrange("(c p) m -> p c m", p=128))
                    for tq8 in range(2):
                        ldma(out=xT8_sb[:, :, 1024 * tq8:1024 * tq8 + 1024],
                             in_=xT8_src[:, :, 1024 * tq8:1024 * tq8 + 1024])
                    for tq in range(2, 4):
                        ldma(out=xT_sb[:, :, 512 * tq:512 * tq + 512],
                             in_=xT_src[:, :, 512 * tq:512 * tq + 512])
                    # off-critical-path loads (after the attention-feeding ones)
                    ldma(out=xown_sb[:],
                         in_=xown_d.ap().rearrange("(j p) d -> p j d", p=128))
                    ldma(out=woutT_sb[:],
                         in_=woutT_d.ap().rearrange("(c p) m -> p c m", p=128))
                    ldma(out=assign_sb[64:72, :, :], in_=assign_d[:])
                    ldma(out=ident_sb[:], in_=ident_d[:])
                    ldma(out=w1T_sb[:],
                         in_=w1T_d.ap().rearrange("(c p) m -> p c m", p=128))
                    if stop_after == "dma":
                        return

                    def emit_q(mi):
                        q_ps = ps_tile(f"q_ps{mi}")
                        for tj in range(2):
                            for c in range(4):
                                nc.tensor.matmul(
                                    q_ps[:, 512 * tj:512 * tj + 512],
                                    wqkvT_sb[:, c, 128 * mi:128 * mi + 128],
                                    xT_sb[:, c, 512 * tj:512 * tj + 512],
                                    start=(c == 0), stop=(c == 3))
                        nc.vector.tensor_scalar(
                            qT_sb[:, mi, :], q_ps[:],
                            bqkv_sb[:, mi:mi + 1], None, OP.add)

                    def emit_k_half(mi, kh):
                        # K^T for head-pair mi, token half kh; evac on DVE
                        # (keeps ACT free for softmax exp)
                        k_ps = ps_tile(f"k_ps{mi}_{kh}")
                        for tj in range(2):
                            for c in range(4):
                                nc.tensor.matmul(
                                    k_ps[:, 512 * tj:512 * tj + 512],
                                    wqkvT_sb[:, c,
                                             512 + 128 * mi:512 + 128 * mi + 128],
                                    xT_sb[:, c, 1024 * kh + 512 * tj:
                                          1024 * kh + 512 * tj + 512],
                                    start=(c == 0), stop=(c == 3))
                        nc.vector.tensor_scalar(
                            kT_sb[:, mi, 1024 * kh:1024 * kh + 1024],
                            k_ps[:], bqkv_sb[:, 4 + mi:5 + mi], None,
                            OP.add)

                    def emit_k(mi):
                        emit_k_half(mi, 0)
                        emit_k_half(mi, 1)

                    def emit_v(ti):
                        # fp8 DoubleRow: two 128-feature contraction chunks
                        # per instruction (V tolerates fp8 inputs well; the
                        # attention average washes element noise out)
                        v_ps = ps_tile(f"v_ps{ti}")
                        for cp in range(2):
                            nc.tensor.matmul(
                                v_ps[:, 0:512],
                                xT8_sb[:, 2 * cp:2 * cp + 2,
                                       128 * ti:128 * ti + 128],
                                wv8_sb[:, 2 * cp:2 * cp + 2, :],
                                start=(cp == 0), stop=(cp == 1),
                                perf_mode=PM.DoubleRow)
                        v_dst = vext_sb[:, ti, :].rearrange(
                            "p (h e) -> p h e", e=VW)[:, :, 0:HD]
                        v_src = v_ps[:, 0:512].rearrange("p (h e) -> p h e", e=HD)
                        if flags["bv"]:
                            nc.vector.tensor_tensor(
                                v_dst, v_src,
                                bc["bv"][:].rearrange("p (h e) -> p h e", e=HD),
                                OP.add)
                        else:
                            nc.vector.tensor_copy(v_dst, v_src)

                    def attn_step(hp, ktp, hh, c_ps, ppool):
                        # scores -> exp -> ctx for one (head-pair, k-tile
                        # PAIR, hh). exp emits fp8 attention weights for the
                        # two k-tiles side by side; ctx then contracts both
                        # in one DoubleRow matmul per 512-token column chunk
                        # (2 fp8 contraction rows per PE cell per cycle).
                        # Softmax renormalizes by the sum of the same fp8
                        # weights (one-hot V columns), so weight quantization
                        # largely cancels.
                        h = 2 * hp + hh
                        p_sb = ppool.tile([128, 2, 1024], FP8, tag="p",
                                          name=f"p{hp}_{ktp}_{hh}")
                        for ki in range(2):
                            kt = 2 * ktp + ki
                            s_ps = ps_tile(f"s_ps{hp}_{ktp}_{hh}_{ki}")
                            lhsT = kT_sb[64 * hh:64 * hh + 64, hp,
                                         128 * kt:128 * kt + 128]
                            for tj in range(2):
                                nc.tensor.matmul(
                                    s_ps[:, 512 * tj:512 * tj + 512],
                                    lhsT,
                                    qT_sb[64 * hh:64 * hh + 64, hp,
                                          512 * tj:512 * tj + 512],
                                    start=True, stop=True)
                            nc.scalar.activation(out=p_sb[:, ki, :],
                                                 in_=s_ps[:], func=AF.Exp)
                        lhsT = vext_sb[:].rearrange(
                            "p t w -> p t w")[:, 2 * ktp:2 * ktp + 2,
                                              VW * h:VW * h + VW]
                        for tj in range(2):
                            nc.tensor.matmul(
                                c_ps[0:VW, 1024 * hh + 512 * tj:
                                     1024 * hh + 512 * tj + 512],
                                lhsT,
                                p_sb[:, :, 512 * tj:512 * tj + 512],
                                start=(ktp == 0), stop=(ktp == 7),
                                perf_mode=PM.DoubleRow)

                    def evac_attn(hp, c_ps):
                        # ctx^T rows 0:64 -> actT. Head 2hp+hh's row-sum sits
                        # on psum row 64+2hp+hh of column half hh (one-hot V
                        # column; other heads' rows are zero there, so the
                        # aligned 8-row block accumulates cleanly).
                        for hh in range(2):
                            nc.vector.tensor_copy(
                                actT_sb[64 * hh:64 * hh + 64, hp, :],
                                c_ps[0:64, 1024 * hh:1024 * hh + SQ])
                            if hp == 0 and hh == 0:
                                nc.vector.tensor_copy(
                                    rsum_sb[64:72, :], c_ps[64:72, 0:SQ])
                            else:
                                nc.vector.tensor_tensor(
                                    rsum_sb[64:72, :], rsum_sb[64:72, :],
                                    c_ps[64:72, 1024 * hh:1024 * hh + SQ],
                                    OP.add)

                    # PE p-state warmup: the array clocks 0.65->2.4GHz
                    # only after ~3us of continuous work; run throwaway
                    # matmuls on a zeroed scratch tile while the first xT/w
                    # DMAs are in flight so Q/K/scores start at full clock.
                    scr = pha.tile([128, 512], BF16, name="scr_sb")
                    nc.gpsimd.memset(scr[:], 0.0)
                    for wu in range(8):
                        w_ps = ps_tile(f"wu{wu}")
                        nc.tensor.matmul(w_ps[:, 0:512], scr[:, 0:128],
                                         scr[:], start=True, stop=True)
                    # Minimal prefix before the first softmax exp: Q(mi0)
                    # and K(0, token-half 0) — everything else (Q1-3, K0h1,
                    # V, K1) rides the PE queue interleaved under hp0's exp
                    # wall. hp0's first 8 k-tiles live in token half 0.
                    emit_q(0)
                    emit_k_half(0, 0)
                    if stop_after == "qkv":
                        for mi in range(1, 4):
                            emit_q(mi)
                        emit_k_half(0, 1)
                        for ti in range(16):
                            emit_v(ti)
                        for mi in range(1, 4):
                            emit_k(mi)
                        return

                    def norm_hp(hp):
                        # per-head-pair softmax denominator. rsum rows for
                        # this hp are final after evac_attn(hp) (later adds
                        # only contribute zeros); stale rows of other pairs
                        # are masked by zeros in assign. The broadcast matmul
                        # runs on the RAW sums; the reciprocal is taken on
                        # the partition-0-aligned broadcast result (single
                        # partitions 65.. are not engine-addressable).
                        n_ps = ps_tile(f"n_ps{hp}")
                        for tj in range(2):
                            nc.tensor.matmul(
                                n_ps[:, 512 * tj:512 * tj + 512],
                                assign_sb[64:72, hp, :],
                                rsum_sb[64:72, 512 * tj:512 * tj + 512],
                                start=True, stop=True)
                        nrec = work.tile([128, SQ], F32, tag="nr",
                                         name=f"nrec{hp}")
                        nc.vector.reciprocal(nrec[:], n_ps[:, 0:SQ])
                        with nc.allow_low_precision(
                                reason="attention weights tolerate bf16"):
                            nc.vector.tensor_tensor(
                                actT_sb[:, hp, :], actT_sb[:, hp, :],
                                nrec[:], OP.mult)

                    c_ps0 = ps_big("c_ps0")
                    for ktp in range(8):
                        emit_v(2 * ktp)
                        emit_v(2 * ktp + 1)
                        attn_step(0, ktp, 0, c_ps0, pp)
                        attn_step(0, ktp, 1, c_ps0, pp)
                        if ktp < 3:
                            emit_q(ktp + 1)
                        elif ktp == 3:
                            emit_k_half(0, 1)
                        elif ktp in (4, 6):
                            emit_k_half(1, (ktp - 4) // 2)
                    evac_attn(0, c_ps0)
                    for hp in range(1, 4):
                        c_ps = ps_big(f"c_ps{hp}")
                        for ktp in range(8):
                            attn_step(hp, ktp, 0, c_ps, pp)
                            attn_step(hp, ktp, 1, c_ps, pp)
                            if hp < 3 and ktp in (2, 5):
                                emit_k_half(hp + 1, (ktp - 2) // 3)
                            elif ktp == 1:
                                norm_hp(hp - 1)
                        evac_attn(hp, c_ps)
                    norm_hp(3)
                    # keep the PE array clocked through the evac/normalize
                    # DVE chain (an idle PE drops to 0.65GHz and would crawl
                    # through the first attn-out matmuls while re-ramping)
                    for wu in range(8):
                        w_ps = ps_tile(f"wt{wu}")
                        nc.tensor.matmul(w_ps[:, 0:512], scr[:, 0:128],
                                         scr[:], start=True, stop=True)



            if stop_after == "attn":
                return
            # ---- attn_out (natural) + LN1 -> x2 (in place over x_own) ----
            def layer_norm(j, acc_ps, resid_ap, out_ap, pre_b, g, bt):
                z = work.tile([128, D], F32, tag="z", name=f"z{j}")
                nc.vector.tensor_tensor(z[:], resid_ap, acc_ps, OP.add)
                if pre_b is not None:
                    nc.vector.tensor_tensor(z[:], z[:], pre_b[:], OP.add)
                st = work.tile([128, 6], F32, tag="st", name=f"st{j}")
                nc.vector.bn_stats(out=st[:], in_=z[:])
                mv = work.tile([128, 2], F32, tag="mv", name=f"mv{j}")
                nc.vector.bn_aggr(out=mv[:], in_=st[:])
                sd = work.tile([128, 1], F32, tag="sd", name=f"sd{j}")
                nc.scalar.activation(out=sd[:], in_=mv[:, 1:2], func=AF.Sqrt,
                                     bias=eps_sb[:], scale=1.0)
                nc.vector.reciprocal(sd[:], sd[:])
                nc.vector.tensor_scalar(out_ap, z[:], mv[:, 0:1], sd[:],
                                        OP.subtract, OP.mult)
                if g is not None:
                    nc.vector.tensor_tensor(out_ap, out_ap, g[:], OP.mult)
                if bt is not None:
                    nc.vector.tensor_tensor(out_ap, out_ap, bt[:], OP.add)

            if True:
                for j in range(8):
                    a_ps = ps_tile(f"a_ps{j}")
                    for c in range(4):
                        nc.tensor.matmul(a_ps[:, 0:512],
                                         actT_sb[:, c, 128 * j:128 * j + 128],
                                         woutT_sb[:, c, :],
                                         start=(c == 0), stop=(c == 3))
                    layer_norm(j, a_ps[:, 0:512], xown_sb[:, j, :],
                               xown_sb[:, j, :],
                               bc.get("bout"), bc.get("g1"), bc.get("bt1"))

                if stop_after == "ln1":
                    return
                # ============ phase 3: FFN + LN2 ============
                # x2^T via PE transposes (into actT, reusing the ctx^T tile)
                for i in range(4):
                    t_ps = ps_tile(f"t_ps{i}", F32R)
                    for j in range(8):
                        nc.tensor.transpose(t_ps[:, 128 * j:128 * j + 128],
                                            xown_sb[:, j, 128 * i:128 * i + 128],
                                            ident_sb[:])
                    with nc.allow_low_precision(
                            reason="x2 feeds fp8 DoubleRow FFN matmuls"):
                        nc.vector.tensor_copy(actT8_sb[:, i, :], t_ps[:])

                with tc.tile_pool(name="hp_", bufs=1) as hpool, \
                     tc.tile_pool(name="w2p", bufs=1) as w2p:
                    w2T_sb = w2p.tile([128, 16, D], BF16, name="w2T_sb")
                    for c in range(0, 16, 4):
                        dma(out=w2T_sb[:, c:c + 4, :],
                            in_=w2T_d.ap().rearrange("(c p) m -> p c m",
                                                     p=128)[:, c:c + 4, :])
                    hT_sb = hpool.tile([128, 16, SQ], BF16, name="hT_sb")

                    def ffn1_m(m):
                        f_ps = ps_tile(f"f_ps{m}")
                        for tj in range(2):
                            for cp in range(2):
                                nc.tensor.matmul(
                                    f_ps[:, 512 * tj:512 * tj + 512],
                                    w1T_sb[:, 2 * cp:2 * cp + 2,
                                           128 * m:128 * m + 128],
                                    actT8_sb[:, 2 * cp:2 * cp + 2,
                                             512 * tj:512 * tj + 512],
                                    start=(cp == 0), stop=(cp == 1),
                                    perf_mode=PM.DoubleRow)
                        nc.scalar.activation(out=hT_sb[:, m, :], in_=f_ps[:],
                                             func=AF.Gelu,
                                             bias=b1_sb[:, m:m + 1], scale=1.0)

                    def ffn2_fc(y_ps, wave, fc):
                        for jj in range(4):
                            j = 4 * wave + jj
                            nc.tensor.matmul(
                                y_ps[:, 512 * jj:512 * jj + 512],
                                hT_sb[:, fc, 128 * j:128 * j + 128],
                                w2T_sb[:, fc, :],
                                start=(fc == 0), stop=(fc == 15))

                    def ln2_wave(y_ps, wave):
                        for jj in range(4):
                            j = 4 * wave + jj
                            o_sb = work.tile([128, D], F32, tag="o",
                                             name=f"o{j}")
                            layer_norm(8 + j, y_ps[:, 512 * jj:512 * jj + 512],
                                       xown_sb[:, j, :], o_sb[:],
                                       bc.get("b2"), bc.get("g2"),
                                       bc.get("bt2"))
                            dma(out=out_d[128 * j:128 * j + 128, :],
                                in_=o_sb[:])

                    for m in range(16):
                        ffn1_m(m)
                    if stop_after == "ffn1":
                        return
                    # plain per-token-block FFN2: each block's 16-chunk psum
                    # accumulation chases the gelu stream naturally (fc<m
                    # chunks run while later gelus are still in flight)
                    for j in range(8):
                        y_ps = ps_tile(f"y_ps{j}")
                        for fc in range(16):
                            nc.tensor.matmul(y_ps[:, 0:512],
                                             hT_sb[:, fc, 128 * j:128 * j + 128],
                                             w2T_sb[:, fc, :],
                                             start=(fc == 0), stop=(fc == 15))
                        o_sb = work.tile([128, D], F32, tag="o", name=f"o{j}")
                        layer_norm(8 + j, y_ps[:, 0:512], xown_sb[:, j, :],
                                   o_sb[:],
                                   bc.get("b2"), bc.get("g2"), bc.get("bt2"))
                        dma(out=out_d[128 * j:128 * j + 128, :], in_=o_sb[:])

    # KERNEL_UNROLL: bodies per For_i iteration. The loop's per-iteration
    # all-engine barrier + semaphore reset is measurement scaffolding, not
    # kernel work; unrolling amortizes it so the measured slope approaches
    # the true back-to-back kernel time.
    unroll = int(os.environ.get("KERNEL_UNROLL", "1"))
    with tile.TileContext(nc) as tc:
        if reps == 1:
            body(tc)
        else:
            assert reps % unroll == 0, (reps, unroll)
            with tc.For_i(0, reps // unroll):
                for _u in range(unroll):
                    body(tc)
    return nc


_NC_CACHE = {}


def _get_nc(flags):
    key = (tuple(sorted(flags.items())),
           os.environ.get("KERNEL_STOP_AFTER", ""),
           os.environ.get("KERNEL_REPS", "1"),
           os.environ.get("KERNEL_UNROLL", "1"))
    if key not in _NC_CACHE:
        nc = bacc.Bacc("TRN2", target_bir_lowering=False, debug=False)
        _emit(nc, flags)
        nc.compile()
        _NC_CACHE[key] = nc
    return _NC_CACHE[key]


LAST_RESULTS = None


def make_in_maps(x, in_proj_w, in_proj_b, out_w, out_b, ln1_g, ln1_b, ln2_g,
                 ln2_b, ff_w1, ff_b1, ff_w2, ff_b2):
    x = np.asarray(x, dtype=np.float32)
    scale = np.float32(1.0 / np.sqrt(HD))

    wqkvT_f = np.ascontiguousarray(np.asarray(in_proj_w, np.float32).T)  # (D, 3D)
    wqkvT_f[:, :D] *= scale
    wqkvT = np.ascontiguousarray(wqkvT_f[:, :2 * D]).astype(ml_dtypes.bfloat16)
    wv8 = np.ascontiguousarray(wqkvT_f[:, 2 * D:]).astype(ml_dtypes.float8_e4m3fn)
    bqkv = np.asarray(in_proj_b, np.float32).copy()
    bqkv[:D] *= scale
    bqkv_pp = np.ascontiguousarray(bqkv.reshape(12, 128).T)
    woutT = np.ascontiguousarray(np.asarray(out_w, np.float32).T).astype(
        ml_dtypes.bfloat16)
    w1T = np.ascontiguousarray(np.asarray(ff_w1, np.float32).T).astype(
        ml_dtypes.float8_e4m3fn)
    b1_pp = np.ascontiguousarray(np.asarray(ff_b1, np.float32).reshape(16, 128).T)
    w2T = np.ascontiguousarray(np.asarray(ff_w2, np.float32).T).astype(
        ml_dtypes.bfloat16)

    assign = np.zeros((8, 4, 128), np.float32)
    for h in range(8):
        i = h // 2
        lo = 64 * (h % 2)
        assign[h, i, lo:lo + 64] = 1.0
    ident = np.eye(128, dtype=np.float32)

    bv = bqkv[2 * D:3 * D]
    vecs = np.stack([
        bv,
        np.asarray(out_b, np.float32),
        np.asarray(ff_b2, np.float32),
        np.asarray(ln1_g, np.float32),
        np.asarray(ln1_b, np.float32),
        np.asarray(ln2_g, np.float32),
        np.asarray(ln2_b, np.float32),
    ]).astype(np.float32)

    flags = {
        "bv": bool(np.any(bv != 0)),
        "bout": bool(np.any(vecs[1] != 0)),
        "b2": bool(np.any(vecs[2] != 0)),
        "g1": bool(np.any(vecs[3] != 1)),
        "bt1": bool(np.any(vecs[4] != 0)),
        "g2": bool(np.any(vecs[5] != 1)),
        "bt2": bool(np.any(vecs[6] != 0)),
    }

    in_maps = []
    for c in range(N_CORES):
        b, hh = c // 2, c % 2
        xb = x[b]
        xT = np.ascontiguousarray(xb.T) if hh == 0 else \
            np.ascontiguousarray(np.roll(xb.T, -SQ, axis=1))
        in_maps.append({
            "xT": xT.astype(ml_dtypes.bfloat16),
            "xT8": xT.astype(ml_dtypes.float8_e4m3fn), "wv8": wv8,
            "x_own": np.ascontiguousarray(xb[SQ * hh:SQ * (hh + 1)]),
            "wqkvT": wqkvT, "bqkv_pp": bqkv_pp, "woutT": woutT,
            "w1T": w1T, "b1_pp": b1_pp, "w2T": w2T,
            "assign": assign, "ident": ident, "vecs": vecs,
        })
    return in_maps, flags


def kernel(x, in_proj_w, in_proj_b, out_w, out_b, ln1_g, ln1_b, ln2_g, ln2_b,
           ff_w1, ff_b1, ff_w2, ff_b2):
    global LAST_RESULTS
    in_maps, flags = make_in_maps(
        x, in_proj_w, in_proj_b, out_w, out_b, ln1_g, ln1_b, ln2_g, ln2_b,
        ff_w1, ff_b1, ff_w2, ff_b2)
    nc = _get_nc(flags)
    res = run_bass_kernel_spmd(
        nc, in_maps, core_ids=list(range(N_CORES)),
        trace=bool(int(os.environ.get("BASS_KERNEL_TRACE", "0"))))
    LAST_RESULTS = res

    out = np.empty((B, S, D), np.float32)
    for c in range(N_CORES):
        b, hh = c // 2, c % 2
        out[b, SQ * hh:SQ * (hh + 1)] = res.results[c]["out"]
    return out



# revision 12
# speedup vs baseline: 1.2547x; 1.0492x over previous
"""Trainium2 Bass kernel for a dense transformer AttentionBlock.

Problem (fixed shapes): B=4, S=2048, D=512, H=8 heads (HD=64), FFN hidden 2048.
  qkv = x @ in_proj_w.T + b ; attn = softmax(q k^T / sqrt(64)) ; ctx = attn @ v
  x1 = LN(x + ctx @ out_w.T + out_b) ; out = LN(x1 + gelu(x1 @ w1.T + b1) @ w2.T + b2)

Sharding: 8 cores, zero collectives. Core c handles batch b=c//2, sequence half
h=c%2 (1024 query tokens). K/V are computed redundantly for the full 2048-token
sequence of the batch on both cores of a pair. One SPMD program for all cores:
for odd cores the host rolls x^T by -1024 columns so the core's own query
tokens always sit at columns [0,1024) (k-token order is irrelevant to softmax).

The kernel is scheduled around the softmax-exp wall: exp of the 8*2048*1024
score matrix runs only on the Activation engine (~128 x 1us instructions) and
is the binding resource of the attention phase, so everything else is arranged
to hide beneath it:
  - minimal prefix (Q head-pair 0, K half 0) starts the first exp ~12us in;
    V / remaining Q / K projections ride the PE queue interleaved under the
    head-pair-0 exp stream, with per-head-pair softmax normalization spaced
    into the NEXT pair's window so boundary scores never wait on a DVE chain.
  - an idle PE drops to 0.65GHz and needs ~3us to re-ramp to 2.4GHz, so the
    schedule never lets PE drain: throwaway warm-up matmuls cover the initial
    DMA wait.
Dtypes: weights/activations feeding matmuls are host-cast bf16 (halves DMA +
SBUF; PE rate is identical to f32r). Three paths run fp8e4m3 with DoubleRow
matmuls (2 contraction rows/cell/cycle): the V projection, attn @ V (exp
emits fp8 weights; the row-sum — via one-hot columns appended to V, landing
on psum rows 64:72 — is computed from the SAME quantized weights, so softmax
renormalization cancels the quantization almost exactly: measured +0e0 vs
bf16), and FFN1 (x2/w1, ~9e-3 rel err, the dominant error term). FFN2 stays
bf16: both FFN layers in fp8 measured 1.47e-2 — too close to the 2e-2 gate.
The residual/LayerNorm path stays f32.

Timing support: KERNEL_REPS>1 wraps the body in a tc.For_i HARDWARE loop so
program size stays constant while device work scales linearly; test.py
extracts true device time as the slope of wall time vs reps (per-call axon
dispatch overhead is ~2-3s, noisy, and grows with program size, so a
Python-unrolled rep loop measures the host, not the kernel).
"""

import os
import numpy as np
import ml_dtypes
from contextlib import ExitStack

import concourse.bass as bass
import concourse.mybir as mybir
import concourse.tile as tile
from concourse import bacc
from concourse.bass_utils import run_bass_kernel_spmd

F32 = mybir.dt.float32
F32R = mybir.dt.float32r
BF16 = mybir.dt.bfloat16
FP8 = mybir.dt.float8e4
PM = mybir.MatmulPerfMode
AF = mybir.ActivationFunctionType
OP = mybir.AluOpType

B, S, D, H = 4, 2048, 512, 8
HD = D // H          # 64
F = 4 * D            # 2048
SQ = S // 2          # 1024 own query tokens per core
EPS = 1e-5
N_CORES = 8

# vext: per head 72 columns = [v(64) | 8 filler]; ones at col 72*h + 64 + h
VW = 72
VEXT_W = H * VW      # 576


def _emit(nc, flags):
    """Emit the whole per-core program. flags: dict of bools for optional ops.
    KERNEL_STOP_AFTER in {qkv, attn, ln1, ffn1} truncates for cost analysis.
    KERNEL_REPS>1 wraps the body in a tc.For_i HARDWARE loop: the program size
    stays constant while device work scales linearly, so wall-time deltas
    between two reps values isolate true device execution time (per-call axon
    dispatch overhead is large, noisy, and scales with program size — a
    Python-unrolled rep loop measures that overhead, not the kernel)."""
    stop_after = os.environ.get("KERNEL_STOP_AFTER", "")
    reps = int(os.environ.get("KERNEL_REPS", "1"))
    # ---- DRAM parameters ----
    xown_d = nc.declare_dram_parameter("x_own", [SQ, D], F32R, isOutput=False)
    wqk8_d = nc.declare_dram_parameter("wqk8", [D, 2 * D], FP8, isOutput=False)
    xT8_d = nc.declare_dram_parameter("xT8", [D, S], FP8, isOutput=False)
    wv8_d = nc.declare_dram_parameter("wv8", [D, D], FP8, isOutput=False)
    bqkv_d = nc.declare_dram_parameter("bqkv_pp", [128, 12], F32, isOutput=False)
    woutT_d = nc.declare_dram_parameter("woutT", [D, D], BF16, isOutput=False)
    w1T_d = nc.declare_dram_parameter("w1T", [D, F], FP8, isOutput=False)
    b1_d = nc.declare_dram_parameter("b1_pp", [128, 16], F32, isOutput=False)
    w2T_d = nc.declare_dram_parameter("w2T", [F, D], BF16, isOutput=False)
    assign_d = nc.declare_dram_parameter("assign", [8, 4, 128], F32R, isOutput=False)
    ident_d = nc.declare_dram_parameter("ident", [128, 128], F32R, isOutput=False)
    vecs_d = nc.declare_dram_parameter("vecs", [7, D], F32, isOutput=False)
    out_d = nc.declare_dram_parameter("out", [SQ, D], F32, isOutput=True)

    VEC_ROW = {"bv": 0, "bout": 1, "b2": 2, "g1": 3, "bt1": 4, "g2": 5, "bt2": 6}

    dma = nc.gpsimd.dma_start      # stores / misc (SWDGE on Pool)
    ldma = nc.sync.dma_start       # loads (HWDGE issued from idle SP engine)

    def bcast_row(pool, name, row):
        t = pool.tile([128, D], F32, tag=f"bc_{name}", name=f"bc_{name}")
        src = vecs_d[row]  # (D,)
        src_b = bass.AP(tensor=src.tensor, offset=src.offset,
                        ap=[[0, 128]] + list(src.ap))
        dma(out=t[:], in_=src_b)
        return t

    def body(tc):
        with ExitStack() as es:
            persist = es.enter_context(tc.tile_pool(name="persist", bufs=1))
            work = es.enter_context(tc.tile_pool(name="work", bufs=4))
            psum = es.enter_context(tc.tile_pool(name="psum", bufs=2, space="PSUM"))
            xo = es.enter_context(tc.tile_pool(name="xo", bufs=1))
            shr = es.enter_context(tc.tile_pool(name="shr", bufs=1))
            w1p = es.enter_context(tc.tile_pool(name="w1p", bufs=1))

            def ps_big(nm):
                # 4-bank ctx accumulator (one buffer)
                return psum.tile([128, 2048], F32, tag="c", name=nm, bufs=1)

            def ps_tile(nm, dt=F32):
                # 2-bank double-buffered working psum
                return psum.tile([128, 1024], dt, tag="s", name=nm, bufs=2)

            # ---- tiny persistent tensors (cheap DMAs; big loads below) ----
            bqkv_sb = persist.tile([128, 12], F32, name="bqkv_sb")
            ldma(out=bqkv_sb[:], in_=bqkv_d[:])
            b1_sb = persist.tile([128, 16], F32, name="b1_sb")
            ldma(out=b1_sb[:], in_=b1_d[:])
            eps_sb = persist.tile([128, 1], F32, name="eps_sb")
            nc.vector.memset(eps_sb[:], EPS)
            bc = {}
            for nm in ("bv", "bout", "b2", "g1", "bt1", "g2", "bt2"):
                if flags[nm]:
                    bc[nm] = bcast_row(persist, nm, VEC_ROW[nm])
            # tiles whose loads are deferred off the critical DMA path
            woutT_sb = persist.tile([128, 4, D], BF16, name="woutT_sb")
            assign_sb = persist.tile([128, 4, 128], F32R, name="assign_sb")
            ident_sb = persist.tile([128, 128], F32R, name="ident_sb")
            xown_sb = xo.tile([128, 8, D], F32R, name="xown_sb")
            # shared feature-major activation tile: ctx^T, later x2^T
            actT_sb = shr.tile([128, 4, SQ], BF16, name="actT_sb")
            # FFN w1, prefetched during attention
            w1T_sb = w1p.tile([128, 4, F], FP8, name="w1T_sb")
            actT8_sb = w1p.tile([128, 4, SQ], FP8, name="actT8_sb")

            with tc.tile_pool(name="qk", bufs=1) as qk:
                qT_sb = qk.tile([128, 4, SQ], BF16, name="qT_sb")
                kT_sb = qk.tile([128, 4, S], BF16, name="kT_sb")
                vext_sb = qk.tile([128, 16, VEXT_W], FP8, name="vext_sb")
                # vext filler: zero cols 64:72 per head, then 1.0 at col 64+h
                # (row-sum one-hot). Pool memsets, no DMA traffic.
                vfill = vext_sb[:].rearrange("p t (h w) -> p t h w", w=VW)
                nc.gpsimd.memset(vfill[:, :, :, HD:VW], 0.0)
                for h in range(H):
                    nc.gpsimd.memset(vfill[:, :, h, HD + h:HD + h + 1], 1.0)

                rsum_sb = qk.tile([128, SQ], F32R, name="rsum_sb")  # 64:72
                with tc.tile_pool(name="pp", bufs=4) as pp, \
                     tc.tile_pool(name="pha", bufs=1) as pha:
                    wqk8_sb = pha.tile([128, 4, 2 * D], FP8, name="wqk8_sb")
                    xT8_sb = pha.tile([128, 4, S], FP8, name="xT8_sb")
                    wv8_sb = pha.tile([128, 4, D], FP8, name="wv8_sb")

                    # ---- load order = need order; everything feeding Q/K/V
                    # is fp8 now, the bf16 x/wqkv copies are gone entirely
                    xT8_src = xT8_d.ap().rearrange("(c p) t -> p c t", p=128)
                    ldma(out=xT8_sb[:, :, 0:1024], in_=xT8_src[:, :, 0:1024])
                    for c in range(4):   # Q weight cols (first exp needs them)
                        ldma(out=wqk8_sb[:, c, 0:512],
                             in_=wqk8_d[128 * c:128 * c + 128, 0:512])
                    for c in range(4):   # K weight cols
                        ldma(out=wqk8_sb[:, c, 512:1024],
                             in_=wqk8_d[128 * c:128 * c + 128, 512:1024])
                    ldma(out=xT8_sb[:, :, 1024:2048],
                         in_=xT8_src[:, :, 1024:2048])
                    ldma(out=wv8_sb[:],
                         in_=wv8_d.ap().rearrange("(c p) m -> p c m", p=128))
                    # off-critical-path loads
                    ldma(out=xown_sb[:],
                         in_=xown_d.ap().rearrange("(j p) d -> p j d", p=128))
                    ldma(out=woutT_sb[:],
                         in_=woutT_d.ap().rearrange("(c p) m -> p c m", p=128))
                    ldma(out=assign_sb[64:72, :, :], in_=assign_d[:])
                    ldma(out=ident_sb[:], in_=ident_d[:])
                    ldma(out=w1T_sb[:],
                         in_=w1T_d.ap().rearrange("(c p) m -> p c m", p=128))
                    if stop_after == "dma":
                        return

                    def emit_q(mi):
                        # fp8 DoubleRow: projection quantization washes
                        # out through softmax normalization (measured +0e0)
                        q_ps = ps_tile(f"q_ps{mi}")
                        for tj in range(2):
                            for cp in range(2):
                                nc.tensor.matmul(
                                    q_ps[:, 512 * tj:512 * tj + 512],
                                    wqk8_sb[:, 2 * cp:2 * cp + 2,
                                            128 * mi:128 * mi + 128],
                                    xT8_sb[:, 2 * cp:2 * cp + 2,
                                           512 * tj:512 * tj + 512],
                                    start=(cp == 0), stop=(cp == 1),
                                    perf_mode=PM.DoubleRow)
                        nc.vector.tensor_scalar(
                            qT_sb[:, mi, :], q_ps[:],
                            bqkv_sb[:, mi:mi + 1], None, OP.add)

                    def emit_k_half(mi, kh):
                        # K^T for head-pair mi, token half kh; evac on DVE
                        # (keeps ACT free for softmax exp)
                        k_ps = ps_tile(f"k_ps{mi}_{kh}")
                        for tj in range(2):
                            for cp in range(2):
                                nc.tensor.matmul(
                                    k_ps[:, 512 * tj:512 * tj + 512],
                                    wqk8_sb[:, 2 * cp:2 * cp + 2,
                                            512 + 128 * mi:512 + 128 * mi + 128],
                                    xT8_sb[:, 2 * cp:2 * cp + 2,
                                           1024 * kh + 512 * tj:
                                           1024 * kh + 512 * tj + 512],
                                    start=(cp == 0), stop=(cp == 1),
                                    perf_mode=PM.DoubleRow)
                        nc.vector.tensor_scalar(
                            kT_sb[:, mi, 1024 * kh:1024 * kh + 1024],
                            k_ps[:], bqkv_sb[:, 4 + mi:5 + mi], None,
                            OP.add)

                    def emit_k(mi):
                        emit_k_half(mi, 0)
                        emit_k_half(mi, 1)

                    def emit_v(ti):
                        # fp8 DoubleRow: two 128-feature contraction chunks
                        # per instruction (V tolerates fp8 inputs well; the
                        # attention average washes element noise out)
                        v_ps = ps_tile(f"v_ps{ti}")
                        for cp in range(2):
                            nc.tensor.matmul(
                                v_ps[:, 0:512],
                                xT8_sb[:, 2 * cp:2 * cp + 2,
                                       128 * ti:128 * ti + 128],
                                wv8_sb[:, 2 * cp:2 * cp + 2, :],
                                start=(cp == 0), stop=(cp == 1),
                                perf_mode=PM.DoubleRow)
                        v_dst = vext_sb[:, ti, :].rearrange(
                            "p (h e) -> p h e", e=VW)[:, :, 0:HD]
                        v_src = v_ps[:, 0:512].rearrange("p (h e) -> p h e", e=HD)
                        if flags["bv"]:
                            nc.vector.tensor_tensor(
                                v_dst, v_src,
                                bc["bv"][:].rearrange("p (h e) -> p h e", e=HD),
                                OP.add)
                        else:
                            nc.vector.tensor_copy(v_dst, v_src)

                    def attn_step(hp, ktp, hh, c_ps, ppool):
                        # scores -> exp -> ctx for one (head-pair, k-tile
                        # PAIR, hh). exp emits fp8 attention weights for the
                        # two k-tiles side by side; ctx then contracts both
                        # in one DoubleRow matmul per 512-token column chunk
                        # (2 fp8 contraction rows per PE cell per cycle).
                        # Softmax renormalizes by the sum of the same fp8
                        # weights (one-hot V columns), so weight quantization
                        # largely cancels.
                        h = 2 * hp + hh
                        p_sb = ppool.tile([128, 2, 1024], FP8, tag="p",
                                          name=f"p{hp}_{ktp}_{hh}")
                        for ki in range(2):
                            kt = 2 * ktp + ki
                            s_ps = ps_tile(f"s_ps{hp}_{ktp}_{hh}_{ki}")
                            lhsT = kT_sb[64 * hh:64 * hh + 64, hp,
                                         128 * kt:128 * kt + 128]
                            for tj in range(2):
                                nc.tensor.matmul(
                                    s_ps[:, 512 * tj:512 * tj + 512],
                                    lhsT,
                                    qT_sb[64 * hh:64 * hh + 64, hp,
                                          512 * tj:512 * tj + 512],
                                    start=True, stop=True)
                            nc.scalar.activation(out=p_sb[:, ki, :],
                                                 in_=s_ps[:], func=AF.Exp)
                        lhsT = vext_sb[:].rearrange(
                            "p t w -> p t w")[:, 2 * ktp:2 * ktp + 2,
                                              VW * h:VW * h + VW]
                        for tj in range(2):
                            nc.tensor.matmul(
                                c_ps[0:VW, 1024 * hh + 512 * tj:
                                     1024 * hh + 512 * tj + 512],
                                lhsT,
                                p_sb[:, :, 512 * tj:512 * tj + 512],
                                start=(ktp == 0), stop=(ktp == 7),
                                perf_mode=PM.DoubleRow)

                    def evac_attn(hp, c_ps):
                        # ctx^T rows 0:64 -> actT. Head 2hp+hh's row-sum sits
                        # on psum row 64+2hp+hh of column half hh (one-hot V
                        # column; other heads' rows are zero there, so the
                        # aligned 8-row block accumulates cleanly).
                        for hh in range(2):
                            nc.vector.tensor_copy(
                                actT_sb[64 * hh:64 * hh + 64, hp, :],
                                c_ps[0:64, 1024 * hh:1024 * hh + SQ])
                            if hp == 0 and hh == 0:
                                nc.vector.tensor_copy(
                                    rsum_sb[64:72, :], c_ps[64:72, 0:SQ])
                            else:
                                nc.vector.tensor_tensor(
                                    rsum_sb[64:72, :], rsum_sb[64:72, :],
                                    c_ps[64:72, 1024 * hh:1024 * hh + SQ],
                                    OP.add)

                    # PE p-state warmup: the array clocks 0.65->2.4GHz
                    # only after ~3us of continuous work; run throwaway
                    # matmuls on a zeroed scratch tile while the first xT/w
                    # DMAs are in flight so Q/K/scores start at full clock.
                    scr = pha.tile([128, 512], BF16, name="scr_sb")
                    nc.gpsimd.memset(scr[:], 0.0)
                    for wu in range(8):
                        w_ps = ps_tile(f"wu{wu}")
                        nc.tensor.matmul(w_ps[:, 0:512], scr[:, 0:128],
                                         scr[:], start=True, stop=True)
                    # Minimal prefix before the first softmax exp: Q(mi0)
                    # and K(0, token-half 0) — everything else (Q1-3, K0h1,
                    # V, K1) rides the PE queue interleaved under hp0's exp
                    # wall. hp0's first 8 k-tiles live in token half 0.
                    emit_q(0)
                    emit_k_half(0, 0)
                    if stop_after == "qkv":
                        for mi in range(1, 4):
                            emit_q(mi)
                        emit_k_half(0, 1)
                        for ti in range(16):
                            emit_v(ti)
                        for mi in range(1, 4):
                            emit_k(mi)
                        return

                    def norm_hp(hp):
                        # per-head-pair softmax denominator. rsum rows for
                        # this hp are final after evac_attn(hp) (later adds
                        # only contribute zeros); stale rows of other pairs
                        # are masked by zeros in assign. The broadcast matmul
                        # runs on the RAW sums; the reciprocal is taken on
                        # the partition-0-aligned broadcast result (single
                        # partitions 65.. are not engine-addressable).
                        n_ps = ps_tile(f"n_ps{hp}")
                        for tj in range(2):
                            nc.tensor.matmul(
                                n_ps[:, 512 * tj:512 * tj + 512],
                                assign_sb[64:72, hp, :],
                                rsum_sb[64:72, 512 * tj:512 * tj + 512],
                                start=True, stop=True)
                        nrec = work.tile([128, SQ], F32, tag="nr",
                                         name=f"nrec{hp}")
                        nc.vector.reciprocal(nrec[:], n_ps[:, 0:SQ])
                        with nc.allow_low_precision(
                                reason="attention weights tolerate bf16"):
                            nc.vector.tensor_tensor(
                                actT_sb[:, hp, :], actT_sb[:, hp, :],
                                nrec[:], OP.mult)

                    c_ps0 = ps_big("c_ps0")
                    for ktp in range(8):
                        emit_v(2 * ktp)
                        emit_v(2 * ktp + 1)
                        attn_step(0, ktp, 0, c_ps0, pp)
                        attn_step(0, ktp, 1, c_ps0, pp)
                        if ktp < 3:
                            emit_q(ktp + 1)
                        elif ktp == 3:
                            emit_k_half(0, 1)
                        elif ktp in (4, 6):
                            emit_k_half(1, (ktp - 4) // 2)
                    evac_attn(0, c_ps0)
                    for hp in range(1, 4):
                        c_ps = ps_big(f"c_ps{hp}")
                        for ktp in range(8):
                            attn_step(hp, ktp, 0, c_ps, pp)
                            attn_step(hp, ktp, 1, c_ps, pp)
                            if hp < 3 and ktp in (2, 5):
                                emit_k_half(hp + 1, (ktp - 2) // 3)
                            elif ktp == 1:
                                norm_hp(hp - 1)
                        evac_attn(hp, c_ps)
                    norm_hp(3)
                    # keep the PE array clocked through the evac/normalize
                    # DVE chain (an idle PE drops to 0.65GHz and would crawl
                    # through the first attn-out matmuls while re-ramping)
                    for wu in range(8):
                        w_ps = ps_tile(f"wt{wu}")
                        nc.tensor.matmul(w_ps[:, 0:512], scr[:, 0:128],
                                         scr[:], start=True, stop=True)



            if stop_after == "attn":
                return
            # ---- attn_out (natural) + LN1 -> x2 (in place over x_own) ----
            def layer_norm(j, acc_ps, resid_ap, out_ap, pre_b, g, bt):
                z = work.tile([128, D], F32, tag="z", name=f"z{j}")
                nc.vector.tensor_tensor(z[:], resid_ap, acc_ps, OP.add)
                if pre_b is not None:
                    nc.vector.tensor_tensor(z[:], z[:], pre_b[:], OP.add)
                st = work.tile([128, 6], F32, tag="st", name=f"st{j}")
                nc.vector.bn_stats(out=st[:], in_=z[:])
                mv = work.tile([128, 2], F32, tag="mv", name=f"mv{j}")
                nc.vector.bn_aggr(out=mv[:], in_=st[:])
                sd = work.tile([128, 1], F32, tag="sd", name=f"sd{j}")
                nc.scalar.activation(out=sd[:], in_=mv[:, 1:2], func=AF.Sqrt,
                                     bias=eps_sb[:], scale=1.0)
                nc.vector.reciprocal(sd[:], sd[:])
                nc.vector.tensor_scalar(out_ap, z[:], mv[:, 0:1], sd[:],
                                        OP.subtract, OP.mult)
                if g is not None:
                    nc.vector.tensor_tensor(out_ap, out_ap, g[:], OP.mult)
                if bt is not None:
                    nc.vector.tensor_tensor(out_ap, out_ap, bt[:], OP.add)

            if True:
                for j in range(8):
                    a_ps = ps_tile(f"a_ps{j}")
                    for c in range(4):
                        nc.tensor.matmul(a_ps[:, 0:512],
                                         actT_sb[:, c, 128 * j:128 * j + 128],
                                         woutT_sb[:, c, :],
                                         start=(c == 0), stop=(c == 3))
                    layer_norm(j, a_ps[:, 0:512], xown_sb[:, j, :],
                               xown_sb[:, j, :],
                               bc.get("bout"), bc.get("g1"), bc.get("bt1"))

                if stop_after == "ln1":
                    return
                # ============ phase 3: FFN + LN2 ============
                # x2^T via PE transposes (into actT, reusing the ctx^T tile)
                for i in range(4):
                    t_ps = ps_tile(f"t_ps{i}", F32R)
                    for j in range(8):
                        nc.tensor.transpose(t_ps[:, 128 * j:128 * j + 128],
                                            xown_sb[:, j, 128 * i:128 * i + 128],
                                            ident_sb[:])
                    with nc.allow_low_precision(
                            reason="x2 feeds fp8 DoubleRow FFN matmuls"):
                        nc.vector.tensor_copy(actT8_sb[:, i, :], t_ps[:])

                with tc.tile_pool(name="hp_", bufs=1) as hpool, \
                     tc.tile_pool(name="w2p", bufs=1) as w2p:
                    w2T_sb = w2p.tile([128, 16, D], BF16, name="w2T_sb")
                    for c in range(0, 16, 4):
                        dma(out=w2T_sb[:, c:c + 4, :],
                            in_=w2T_d.ap().rearrange("(c p) m -> p c m",
                                                     p=128)[:, c:c + 4, :])
                    hT_sb = hpool.tile([128, 16, SQ], BF16, name="hT_sb")

                    def ffn1_m(m):
                        f_ps = ps_tile(f"f_ps{m}")
                        for tj in range(2):
                            for cp in range(2):
                                nc.tensor.matmul(
                                    f_ps[:, 512 * tj:512 * tj + 512],
                                    w1T_sb[:, 2 * cp:2 * cp + 2,
                                           128 * m:128 * m + 128],
                                    actT8_sb[:, 2 * cp:2 * cp + 2,
                                             512 * tj:512 * tj + 512],
                                    start=(cp == 0), stop=(cp == 1),
                                    perf_mode=PM.DoubleRow)
                        nc.scalar.activation(out=hT_sb[:, m, :], in_=f_ps[:],
                                             func=AF.Gelu,
                                             bias=b1_sb[:, m:m + 1], scale=1.0)

                    def ffn2_fc(y_ps, wave, fc):
                        for jj in range(4):
                            j = 4 * wave + jj
                            nc.tensor.matmul(
                                y_ps[:, 512 * jj:512 * jj + 512],
                                hT_sb[:, fc, 128 * j:128 * j + 128],
                                w2T_sb[:, fc, :],
                                start=(fc == 0), stop=(fc == 15))

                    def ln2_wave(y_ps, wave):
                        for jj in range(4):
                            j = 4 * wave + jj
                            o_sb = work.tile([128, D], F32, tag="o",
                                             name=f"o{j}")
                            layer_norm(8 + j, y_ps[:, 512 * jj:512 * jj + 512],
                                       xown_sb[:, j, :], o_sb[:],
                                       bc.get("b2"), bc.get("g2"),
                                       bc.get("bt2"))
                            dma(out=out_d[128 * j:128 * j + 128, :],
                                in_=o_sb[:])

                    for m in range(16):
                        ffn1_m(m)
                    if stop_after == "ffn1":
                        return
                    # plain per-token-block FFN2: each block's 16-chunk psum
                    # accumulation chases the gelu stream naturally (fc<m
                    # chunks run while later gelus are still in flight)
                    for j in range(8):
                        y_ps = ps_tile(f"y_ps{j}")
                        for fc in range(16):
                            nc.tensor.matmul(y_ps[:, 0:512],
                                             hT_sb[:, fc, 128 * j:128 * j + 128],
                                             w2T_sb[:, fc, :],
                                             start=(fc == 0), stop=(fc == 15))
                        o_sb = work.tile([128, D], F32, tag="o", name=f"o{j}")
                        layer_norm(8 + j, y_ps[:, 0:512], xown_sb[:, j, :],
                                   o_sb[:],
                                   bc.get("b2"), bc.get("g2"), bc.get("bt2"))
                        dma(out=out_d[128 * j:128 * j + 128, :], in_=o_sb[:])

    # KERNEL_UNROLL: bodies per For_i iteration. The loop's per-iteration
    # all-engine barrier + semaphore reset is measurement scaffolding, not
    # kernel work; unrolling amortizes it so the measured slope approaches
    # the true back-to-back kernel time.
    unroll = int(os.environ.get("KERNEL_UNROLL", "1"))
    with tile.TileContext(nc) as tc:
        if reps == 1:
            body(tc)
        else:
            assert reps % unroll == 0, (reps, unroll)
            with tc.For_i(0, reps // unroll):
                for _u in range(unroll):
                    body(tc)
    return nc


_NC_CACHE = {}


def _get_nc(flags):
    key = (tuple(sorted(flags.items())),
           os.environ.get("KERNEL_STOP_AFTER", ""),
           os.environ.get("KERNEL_REPS", "1"),
           os.environ.get("KERNEL_UNROLL", "1"))
    if key not in _NC_CACHE:
        nc = bacc.Bacc("TRN2", target_bir_lowering=False, debug=False)
        _emit(nc, flags)
        nc.compile()
        _NC_CACHE[key] = nc
    return _NC_CACHE[key]


LAST_RESULTS = None


def make_in_maps(x, in_proj_w, in_proj_b, out_w, out_b, ln1_g, ln1_b, ln2_g,
                 ln2_b, ff_w1, ff_b1, ff_w2, ff_b2):
    x = np.asarray(x, dtype=np.float32)
    scale = np.float32(1.0 / np.sqrt(HD))

    wqkvT_f = np.ascontiguousarray(np.asarray(in_proj_w, np.float32).T)  # (D, 3D)
    wqkvT_f[:, :D] *= scale
    wqk8 = np.ascontiguousarray(wqkvT_f[:, :2 * D]).astype(
        ml_dtypes.float8_e4m3fn)
    wv8 = np.ascontiguousarray(wqkvT_f[:, 2 * D:]).astype(ml_dtypes.float8_e4m3fn)
    bqkv = np.asarray(in_proj_b, np.float32).copy()
    bqkv[:D] *= scale
    bqkv_pp = np.ascontiguousarray(bqkv.reshape(12, 128).T)
    woutT = np.ascontiguousarray(np.asarray(out_w, np.float32).T).astype(
        ml_dtypes.bfloat16)
    w1T = np.ascontiguousarray(np.asarray(ff_w1, np.float32).T).astype(
        ml_dtypes.float8_e4m3fn)
    b1_pp = np.ascontiguousarray(np.asarray(ff_b1, np.float32).reshape(16, 128).T)
    w2T = np.ascontiguousarray(np.asarray(ff_w2, np.float32).T).astype(
        ml_dtypes.bfloat16)

    assign = np.zeros((8, 4, 128), np.float32)
    for h in range(8):
        i = h // 2
        lo = 64 * (h % 2)
        assign[h, i, lo:lo + 64] = 1.0
    ident = np.eye(128, dtype=np.float32)

    bv = bqkv[2 * D:3 * D]
    vecs = np.stack([
        bv,
        np.asarray(out_b, np.float32),
        np.asarray(ff_b2, np.float32),
        np.asarray(ln1_g, np.float32),
        np.asarray(ln1_b, np.float32),
        np.asarray(ln2_g, np.float32),
        np.asarray(ln2_b, np.float32),
    ]).astype(np.float32)

    flags = {
        "bv": bool(np.any(bv != 0)),
        "bout": bool(np.any(vecs[1] != 0)),
        "b2": bool(np.any(vecs[2] != 0)),
        "g1": bool(np.any(vecs[3] != 1)),
        "bt1": bool(np.any(vecs[4] != 0)),
        "g2": bool(np.any(vecs[5] != 1)),
        "bt2": bool(np.any(vecs[6] != 0)),
    }

    in_maps = []
    for c in range(N_CORES):
        b, hh = c // 2, c % 2
        xb = x[b]
        xT = np.ascontiguousarray(xb.T) if hh == 0 else \
            np.ascontiguousarray(np.roll(xb.T, -SQ, axis=1))
        in_maps.append({
            "xT8": xT.astype(ml_dtypes.float8_e4m3fn), "wv8": wv8,
            "wqk8": wqk8,
            "x_own": np.ascontiguousarray(xb[SQ * hh:SQ * (hh + 1)]),
            "bqkv_pp": bqkv_pp, "woutT": woutT,
            "w1T": w1T, "b1_pp": b1_pp, "w2T": w2T,
            "assign": assign, "ident": ident, "vecs": vecs,
        })
    return in_maps, flags


def kernel(x, in_proj_w, in_proj_b, out_w, out_b, ln1_g, ln1_b, ln2_g, ln2_b,
           ff_w1, ff_b1, ff_w2, ff_b2):
    global LAST_RESULTS
    in_maps, flags = make_in_maps(
        x, in_proj_w, in_proj_b, out_w, out_b, ln1_g, ln1_b, ln2_g, ln2_b,
        ff_w1, ff_b1, ff_w2, ff_b2)
    nc = _get_nc(flags)
    res = run_bass_kernel_spmd(
        nc, in_maps, core_ids=list(range(N_CORES)),
        trace=bool(int(os.environ.get("BASS_KERNEL_TRACE", "0"))))
    LAST_RESULTS = res

    out = np.empty((B, S, D), np.float32)
    for c in range(N_CORES):
        b, hh = c // 2, c % 2
        out[b, SQ * hh:SQ * (hh + 1)] = res.results[c]["out"]
    return out



# revision 15
# speedup vs baseline: 1.2614x; 1.0054x over previous
"""Trainium2 Bass kernel for a dense transformer AttentionBlock.

Problem (fixed shapes): B=4, S=2048, D=512, H=8 heads (HD=64), FFN hidden 2048.
  qkv = x @ in_proj_w.T + b ; attn = softmax(q k^T / sqrt(64)) ; ctx = attn @ v
  x1 = LN(x + ctx @ out_w.T + out_b) ; out = LN(x1 + gelu(x1 @ w1.T + b1) @ w2.T + b2)

Sharding: 8 cores, zero collectives. Core c handles batch b=c//2, sequence half
h=c%2 (1024 query tokens). K/V are computed redundantly for the full 2048-token
sequence of the batch on both cores of a pair. One SPMD program for all cores:
for odd cores the host rolls x^T by -1024 columns so the core's own query
tokens always sit at columns [0,1024) (k-token order is irrelevant to softmax).

The kernel is scheduled around the softmax-exp wall: exp of the 8*2048*1024
score matrix runs only on the Activation engine (~128 x 1us instructions) and
is the binding resource of the attention phase, so everything else is arranged
to hide beneath it:
  - minimal prefix (Q head-pair 0, K half 0) starts the first exp ~12us in;
    V / remaining Q / K projections ride the PE queue interleaved under the
    head-pair-0 exp stream, with per-head-pair softmax normalization spaced
    into the NEXT pair's window so boundary scores never wait on a DVE chain.
  - an idle PE drops to 0.65GHz and needs ~3us to re-ramp to 2.4GHz, so the
    schedule never lets PE drain: throwaway warm-up matmuls cover the initial
    DMA wait.
Dtypes: weights/activations feeding matmuls are host-cast bf16 (halves DMA +
SBUF; PE rate is identical to f32r). Three paths run fp8e4m3 with DoubleRow
matmuls (2 contraction rows/cell/cycle): the V projection, attn @ V (exp
emits fp8 weights; the row-sum — via one-hot columns appended to V, landing
on psum rows 64:72 — is computed from the SAME quantized weights, so softmax
renormalization cancels the quantization almost exactly: measured +0e0 vs
bf16), and FFN1 (x2/w1, ~9e-3 rel err, the dominant error term). FFN2 stays
bf16: both FFN layers in fp8 measured 1.47e-2 — too close to the 2e-2 gate.
The residual/LayerNorm path stays f32.

Timing support: KERNEL_REPS>1 wraps the body in a tc.For_i HARDWARE loop so
program size stays constant while device work scales linearly; test.py
extracts true device time as the slope of wall time vs reps (per-call axon
dispatch overhead is ~2-3s, noisy, and grows with program size, so a
Python-unrolled rep loop measures the host, not the kernel).
"""

import os
import numpy as np
import ml_dtypes
from contextlib import ExitStack

import concourse.bass as bass
import concourse.mybir as mybir
import concourse.tile as tile
from concourse import bacc
from concourse.bass_utils import run_bass_kernel_spmd

F32 = mybir.dt.float32
F32R = mybir.dt.float32r
BF16 = mybir.dt.bfloat16
FP8 = mybir.dt.float8e4
PM = mybir.MatmulPerfMode
AF = mybir.ActivationFunctionType
OP = mybir.AluOpType

B, S, D, H = 4, 2048, 512, 8
HD = D // H          # 64
F = 4 * D            # 2048
SQ = S // 2          # 1024 own query tokens per core
EPS = 1e-5
N_CORES = 8

# vext: per head 72 columns = [v(64) | 8 filler]; ones at col 72*h + 64 + h
VW = 72
VEXT_W = H * VW      # 576


def _emit(nc, flags):
    """Emit the whole per-core program. flags: dict of bools for optional ops.
    KERNEL_STOP_AFTER in {qkv, attn, ln1, ffn1} truncates for cost analysis.
    KERNEL_REPS>1 wraps the body in a tc.For_i HARDWARE loop: the program size
    stays constant while device work scales linearly, so wall-time deltas
    between two reps values isolate true device execution time (per-call axon
    dispatch overhead is large, noisy, and scales with program size — a
    Python-unrolled rep loop measures that overhead, not the kernel)."""
    stop_after = os.environ.get("KERNEL_STOP_AFTER", "")
    reps = int(os.environ.get("KERNEL_REPS", "1"))
    # ---- DRAM parameters ----
    xown_d = nc.declare_dram_parameter("x_own", [SQ, D], F32R, isOutput=False)
    wqk8_d = nc.declare_dram_parameter("wqk8", [D, 2 * D], FP8, isOutput=False)
    xT8_d = nc.declare_dram_parameter("xT8", [D, S], FP8, isOutput=False)
    wv8_d = nc.declare_dram_parameter("wv8", [D, D], FP8, isOutput=False)
    bqkv_d = nc.declare_dram_parameter("bqkv_pp", [128, 12], F32, isOutput=False)
    woutT_d = nc.declare_dram_parameter("woutT", [D, D], BF16, isOutput=False)
    w1T_d = nc.declare_dram_parameter("w1T", [D, F], FP8, isOutput=False)
    b1_d = nc.declare_dram_parameter("b1_pp", [128, 16], F32, isOutput=False)
    w2T_d = nc.declare_dram_parameter("w2T", [F, D], BF16, isOutput=False)
    assign_d = nc.declare_dram_parameter("assign", [8, 4, 128], F32R, isOutput=False)
    ident_d = nc.declare_dram_parameter("ident", [128, 128], F32R, isOutput=False)
    vecs_d = nc.declare_dram_parameter("vecs", [7, D], F32, isOutput=False)
    out_d = nc.declare_dram_parameter("out", [SQ, D], F32, isOutput=True)

    VEC_ROW = {"bv": 0, "bout": 1, "b2": 2, "g1": 3, "bt1": 4, "g2": 5, "bt2": 6}

    dma = nc.gpsimd.dma_start      # stores / misc (SWDGE on Pool)
    ldma = nc.sync.dma_start       # loads (HWDGE issued from idle SP engine)

    def bcast_row(pool, name, row):
        t = pool.tile([128, D], F32, tag=f"bc_{name}", name=f"bc_{name}")
        src = vecs_d[row]  # (D,)
        src_b = bass.AP(tensor=src.tensor, offset=src.offset,
                        ap=[[0, 128]] + list(src.ap))
        dma(out=t[:], in_=src_b)
        return t

    def body(tc):
        with ExitStack() as es:
            persist = es.enter_context(tc.tile_pool(name="persist", bufs=1))
            work = es.enter_context(tc.tile_pool(name="work", bufs=4))
            psum = es.enter_context(tc.tile_pool(name="psum", bufs=2, space="PSUM"))
            xo = es.enter_context(tc.tile_pool(name="xo", bufs=1))
            shr = es.enter_context(tc.tile_pool(name="shr", bufs=1))
            w1p = es.enter_context(tc.tile_pool(name="w1p", bufs=1))

            def ps_big(nm):
                # 4-bank ctx accumulator (one buffer)
                return psum.tile([128, 2048], F32, tag="c", name=nm, bufs=1)

            def ps_tile(nm, dt=F32):
                # 2-bank double-buffered working psum
                return psum.tile([128, 1024], dt, tag="s", name=nm, bufs=2)

            # ---- tiny persistent tensors (cheap DMAs; big loads below) ----
            bqkv_sb = persist.tile([128, 12], F32, name="bqkv_sb")
            ldma(out=bqkv_sb[:], in_=bqkv_d[:])
            b1_sb = persist.tile([128, 16], F32, name="b1_sb")
            ldma(out=b1_sb[:], in_=b1_d[:])
            eps_sb = persist.tile([128, 1], F32, name="eps_sb")
            nc.vector.memset(eps_sb[:], EPS)
            bc = {}
            for nm in ("bv", "bout", "b2", "g1", "bt1", "g2", "bt2"):
                if flags[nm]:
                    bc[nm] = bcast_row(persist, nm, VEC_ROW[nm])
            # tiles whose loads are deferred off the critical DMA path
            woutT_sb = persist.tile([128, 4, D], BF16, name="woutT_sb")
            assign_sb = persist.tile([128, 4, 128], F32R, name="assign_sb")
            ident_sb = persist.tile([128, 128], F32R, name="ident_sb")
            xown_sb = xo.tile([128, 8, D], F32R, name="xown_sb")
            # shared feature-major activation tile: ctx^T, later x2^T
            actT_sb = shr.tile([128, 4, SQ], BF16, name="actT_sb")
            # FFN w1, prefetched during attention
            w1T_sb = w1p.tile([128, 4, F], FP8, name="w1T_sb")
            actT8_sb = w1p.tile([128, 4, SQ], FP8, name="actT8_sb")

            with tc.tile_pool(name="qk", bufs=1) as qk:
                qT_sb = qk.tile([128, 4, SQ], BF16, name="qT_sb")
                kT_sb = qk.tile([128, 4, S], BF16, name="kT_sb")
                vext_sb = qk.tile([128, 16, VEXT_W], FP8, name="vext_sb")
                # vext filler: zero cols 64:72 per head, then 1.0 at col 64+h
                # (row-sum one-hot). Pool memsets, no DMA traffic.
                vfill = vext_sb[:].rearrange("p t (h w) -> p t h w", w=VW)
                nc.gpsimd.memset(vfill[:, :, :, HD:VW], 0.0)
                for h in range(H):
                    nc.gpsimd.memset(vfill[:, :, h, HD + h:HD + h + 1], 1.0)

                rsum_sb = qk.tile([128, SQ], F32R, name="rsum_sb")  # 64:72
                with tc.tile_pool(name="pp", bufs=4) as pp, \
                     tc.tile_pool(name="pha", bufs=1) as pha:
                    wqk8_sb = pha.tile([128, 4, 2 * D], FP8, name="wqk8_sb")
                    xT8_sb = pha.tile([128, 4, S], FP8, name="xT8_sb")
                    wv8_sb = pha.tile([128, 4, D], FP8, name="wv8_sb")

                    # ---- load order = need order; everything feeding Q/K/V
                    # is fp8 now, the bf16 x/wqkv copies are gone entirely
                    xT8_src = xT8_d.ap().rearrange("(c p) t -> p c t", p=128)
                    ldma(out=xT8_sb[:, :, 0:1024], in_=xT8_src[:, :, 0:1024])
                    for c in range(4):   # Q weight cols (first exp needs them)
                        ldma(out=wqk8_sb[:, c, 0:512],
                             in_=wqk8_d[128 * c:128 * c + 128, 0:512])
                    for c in range(4):   # K weight cols
                        ldma(out=wqk8_sb[:, c, 512:1024],
                             in_=wqk8_d[128 * c:128 * c + 128, 512:1024])
                    ldma(out=xT8_sb[:, :, 1024:2048],
                         in_=xT8_src[:, :, 1024:2048])
                    ldma(out=wv8_sb[:],
                         in_=wv8_d.ap().rearrange("(c p) m -> p c m", p=128))
                    # off-critical-path loads
                    ldma(out=xown_sb[:],
                         in_=xown_d.ap().rearrange("(j p) d -> p j d", p=128))
                    ldma(out=woutT_sb[:],
                         in_=woutT_d.ap().rearrange("(c p) m -> p c m", p=128))
                    ldma(out=assign_sb[64:72, :, :], in_=assign_d[:])
                    ldma(out=ident_sb[:], in_=ident_d[:])
                    ldma(out=w1T_sb[:],
                         in_=w1T_d.ap().rearrange("(c p) m -> p c m", p=128))
                    if stop_after == "dma":
                        return

                    def emit_q(mi):
                        # fp8 DoubleRow: projection quantization washes
                        # out through softmax normalization (measured +0e0)
                        q_ps = ps_tile(f"q_ps{mi}")
                        for tj in range(2):
                            for cp in range(2):
                                nc.tensor.matmul(
                                    q_ps[:, 512 * tj:512 * tj + 512],
                                    wqk8_sb[:, 2 * cp:2 * cp + 2,
                                            128 * mi:128 * mi + 128],
                                    xT8_sb[:, 2 * cp:2 * cp + 2,
                                           512 * tj:512 * tj + 512],
                                    start=(cp == 0), stop=(cp == 1),
                                    perf_mode=PM.DoubleRow)
                        nc.vector.tensor_scalar(
                            qT_sb[:, mi, :], q_ps[:],
                            bqkv_sb[:, mi:mi + 1], None, OP.add)

                    def emit_k_half(mi, kh):
                        # K^T for head-pair mi, token half kh; evac on DVE
                        # (keeps ACT free for softmax exp)
                        k_ps = ps_tile(f"k_ps{mi}_{kh}")
                        for tj in range(2):
                            for cp in range(2):
                                nc.tensor.matmul(
                                    k_ps[:, 512 * tj:512 * tj + 512],
                                    wqk8_sb[:, 2 * cp:2 * cp + 2,
                                            512 + 128 * mi:512 + 128 * mi + 128],
                                    xT8_sb[:, 2 * cp:2 * cp + 2,
                                           1024 * kh + 512 * tj:
                                           1024 * kh + 512 * tj + 512],
                                    start=(cp == 0), stop=(cp == 1),
                                    perf_mode=PM.DoubleRow)
                        nc.vector.tensor_scalar(
                            kT_sb[:, mi, 1024 * kh:1024 * kh + 1024],
                            k_ps[:], bqkv_sb[:, 4 + mi:5 + mi], None,
                            OP.add)

                    def emit_k(mi):
                        emit_k_half(mi, 0)
                        emit_k_half(mi, 1)

                    def emit_v(ti):
                        # fp8 DoubleRow: two 128-feature contraction chunks
                        # per instruction (V tolerates fp8 inputs well; the
                        # attention average washes element noise out)
                        v_ps = ps_tile(f"v_ps{ti}")
                        for cp in range(2):
                            nc.tensor.matmul(
                                v_ps[:, 0:512],
                                xT8_sb[:, 2 * cp:2 * cp + 2,
                                       128 * ti:128 * ti + 128],
                                wv8_sb[:, 2 * cp:2 * cp + 2, :],
                                start=(cp == 0), stop=(cp == 1),
                                perf_mode=PM.DoubleRow)
                        v_dst = vext_sb[:, ti, :].rearrange(
                            "p (h e) -> p h e", e=VW)[:, :, 0:HD]
                        v_src = v_ps[:, 0:512].rearrange("p (h e) -> p h e", e=HD)
                        if flags["bv"]:
                            nc.vector.tensor_tensor(
                                v_dst, v_src,
                                bc["bv"][:].rearrange("p (h e) -> p h e", e=HD),
                                OP.add)
                        else:
                            nc.vector.tensor_copy(v_dst, v_src)

                    def attn_step(hp, ktp, hh, c_ps, ppool):
                        # scores -> exp -> ctx for one (head-pair, k-tile
                        # PAIR, hh). exp emits fp8 attention weights for the
                        # two k-tiles side by side; ctx then contracts both
                        # in one DoubleRow matmul per 512-token column chunk
                        # (2 fp8 contraction rows per PE cell per cycle).
                        # Softmax renormalizes by the sum of the same fp8
                        # weights (one-hot V columns), so weight quantization
                        # largely cancels.
                        h = 2 * hp + hh
                        p_sb = ppool.tile([128, 2, 1024], FP8, tag="p",
                                          name=f"p{hp}_{ktp}_{hh}")
                        for ki in range(2):
                            kt = 2 * ktp + ki
                            s_ps = ps_tile(f"s_ps{hp}_{ktp}_{hh}_{ki}")
                            lhsT = kT_sb[64 * hh:64 * hh + 64, hp,
                                         128 * kt:128 * kt + 128]
                            for tj in range(2):
                                nc.tensor.matmul(
                                    s_ps[:, 512 * tj:512 * tj + 512],
                                    lhsT,
                                    qT_sb[64 * hh:64 * hh + 64, hp,
                                          512 * tj:512 * tj + 512],
                                    start=True, stop=True)
                            nc.scalar.activation(out=p_sb[:, ki, :],
                                                 in_=s_ps[:], func=AF.Exp)
                        lhsT = vext_sb[:].rearrange(
                            "p t w -> p t w")[:, 2 * ktp:2 * ktp + 2,
                                              VW * h:VW * h + VW]
                        for tj in range(2):
                            nc.tensor.matmul(
                                c_ps[0:VW, 1024 * hh + 512 * tj:
                                     1024 * hh + 512 * tj + 512],
                                lhsT,
                                p_sb[:, :, 512 * tj:512 * tj + 512],
                                start=(ktp == 0), stop=(ktp == 7),
                                perf_mode=PM.DoubleRow)

                    def evac_attn(hp, c_ps):
                        # ctx^T rows 0:64 -> actT. Head 2hp+hh's row-sum sits
                        # on psum row 64+2hp+hh of column half hh (one-hot V
                        # column; other heads' rows are zero there, so the
                        # aligned 8-row block accumulates cleanly).
                        for hh in range(2):
                            nc.vector.tensor_copy(
                                actT_sb[64 * hh:64 * hh + 64, hp, :],
                                c_ps[0:64, 1024 * hh:1024 * hh + SQ])
                            if hp == 0 and hh == 0:
                                nc.vector.tensor_copy(
                                    rsum_sb[64:72, :], c_ps[64:72, 0:SQ])
                            else:
                                nc.vector.tensor_tensor(
                                    rsum_sb[64:72, :], rsum_sb[64:72, :],
                                    c_ps[64:72, 1024 * hh:1024 * hh + SQ],
                                    OP.add)

                    # PE p-state warmup: the array clocks 0.65->2.4GHz
                    # only after ~3us of continuous work; run throwaway
                    # matmuls on a zeroed scratch tile while the first xT/w
                    # DMAs are in flight so Q/K/scores start at full clock.
                    scr = pha.tile([128, 512], BF16, name="scr_sb")
                    nc.gpsimd.memset(scr[:], 0.0)
                    for wu in range(8):
                        w_ps = ps_tile(f"wu{wu}")
                        nc.tensor.matmul(w_ps[:, 0:512], scr[:, 0:128],
                                         scr[:], start=True, stop=True)
                    # Minimal prefix before the first softmax exp: Q(mi0)
                    # and K(0, token-half 0) — everything else (Q1-3, K0h1,
                    # V, K1) rides the PE queue interleaved under hp0's exp
                    # wall. hp0's first 8 k-tiles live in token half 0.
                    emit_q(0)
                    emit_k_half(0, 0)
                    if stop_after == "qkv":
                        for mi in range(1, 4):
                            emit_q(mi)
                        emit_k_half(0, 1)
                        for ti in range(16):
                            emit_v(ti)
                        for mi in range(1, 4):
                            emit_k(mi)
                        return

                    def norm_hp(hp):
                        # per-head-pair softmax denominator. rsum rows for
                        # this hp are final after evac_attn(hp) (later adds
                        # only contribute zeros); stale rows of other pairs
                        # are masked by zeros in assign. The broadcast matmul
                        # runs on the RAW sums; the reciprocal is taken on
                        # the partition-0-aligned broadcast result (single
                        # partitions 65.. are not engine-addressable).
                        n_ps = ps_tile(f"n_ps{hp}")
                        for tj in range(2):
                            nc.tensor.matmul(
                                n_ps[:, 512 * tj:512 * tj + 512],
                                assign_sb[64:72, hp, :],
                                rsum_sb[64:72, 512 * tj:512 * tj + 512],
                                start=True, stop=True)
                        nrec = work.tile([128, SQ], F32, tag="nr",
                                         name=f"nrec{hp}")
                        nc.vector.reciprocal(nrec[:], n_ps[:, 0:SQ])
                        with nc.allow_low_precision(
                                reason="attention weights tolerate bf16"):
                            nc.vector.tensor_tensor(
                                actT_sb[:, hp, :], actT_sb[:, hp, :],
                                nrec[:], OP.mult)

                    c_ps0 = ps_big("c_ps0")
                    for ktp in range(8):
                        emit_v(2 * ktp)
                        emit_v(2 * ktp + 1)
                        attn_step(0, ktp, 0, c_ps0, pp)
                        attn_step(0, ktp, 1, c_ps0, pp)
                        if ktp < 3:
                            emit_q(ktp + 1)
                        elif ktp == 3:
                            emit_k_half(0, 1)
                        elif ktp in (4, 6):
                            emit_k_half(1, (ktp - 4) // 2)
                    evac_attn(0, c_ps0)
                    for hp in range(1, 4):
                        c_ps = ps_big(f"c_ps{hp}")
                        for ktp in range(8):
                            attn_step(hp, ktp, 0, c_ps, pp)
                            attn_step(hp, ktp, 1, c_ps, pp)
                            if hp < 3 and ktp in (2, 5):
                                emit_k_half(hp + 1, (ktp - 2) // 3)
                            elif ktp == 1:
                                norm_hp(hp - 1)
                        evac_attn(hp, c_ps)
                    norm_hp(3)
                    # keep the PE array clocked through the evac/normalize
                    # DVE chain (an idle PE drops to 0.65GHz and would crawl
                    # through the first attn-out matmuls while re-ramping)
                    for wu in range(8):
                        w_ps = ps_tile(f"wt{wu}")
                        nc.tensor.matmul(w_ps[:, 0:512], scr[:, 0:128],
                                         scr[:], start=True, stop=True)



            if stop_after == "attn":
                return
            # ---- attn_out (natural) + LN1 -> x2 (in place over x_own) ----
            def layer_norm(j, acc_ps, resid_ap, out_ap, pre_b, g, bt):
                z = work.tile([128, D], F32, tag="z", name=f"z{j}")
                nc.vector.tensor_tensor(z[:], resid_ap, acc_ps, OP.add)
                if pre_b is not None:
                    nc.vector.tensor_tensor(z[:], z[:], pre_b[:], OP.add)
                st = work.tile([128, 6], F32, tag="st", name=f"st{j}")
                nc.vector.bn_stats(out=st[:], in_=z[:])
                mv = work.tile([128, 2], F32, tag="mv", name=f"mv{j}")
                nc.vector.bn_aggr(out=mv[:], in_=st[:])
                sd = work.tile([128, 1], F32, tag="sd", name=f"sd{j}")
                nc.scalar.activation(out=sd[:], in_=mv[:, 1:2], func=AF.Sqrt,
                                     bias=eps_sb[:], scale=1.0)
                nc.vector.reciprocal(sd[:], sd[:])
                nc.vector.tensor_scalar(out_ap, z[:], mv[:, 0:1], sd[:],
                                        OP.subtract, OP.mult)
                if g is not None:
                    nc.vector.tensor_tensor(out_ap, out_ap, g[:], OP.mult)
                if bt is not None:
                    nc.vector.tensor_tensor(out_ap, out_ap, bt[:], OP.add)

            if True:
                for j in range(8):
                    a_ps = ps_tile(f"a_ps{j}")
                    for c in range(4):
                        nc.tensor.matmul(a_ps[:, 0:512],
                                         actT_sb[:, c, 128 * j:128 * j + 128],
                                         woutT_sb[:, c, :],
                                         start=(c == 0), stop=(c == 3))
                    layer_norm(j, a_ps[:, 0:512], xown_sb[:, j, :],
                               xown_sb[:, j, :],
                               bc.get("bout"), bc.get("g1"), bc.get("bt1"))

                if stop_after == "ln1":
                    return
                # ============ phase 3: FFN + LN2 ============
                # x2^T via PE transposes (into actT, reusing the ctx^T tile)
                for i in range(4):
                    t_ps = ps_tile(f"t_ps{i}", F32R)
                    for j in range(8):
                        nc.tensor.transpose(t_ps[:, 128 * j:128 * j + 128],
                                            xown_sb[:, j, 128 * i:128 * i + 128],
                                            ident_sb[:])
                    with nc.allow_low_precision(
                            reason="x2 feeds fp8 DoubleRow FFN matmuls"):
                        nc.vector.tensor_copy(actT8_sb[:, i, :], t_ps[:])

                with tc.tile_pool(name="hp_", bufs=1) as hpool, \
                     tc.tile_pool(name="w2p", bufs=1) as w2p:
                    w2T_sb = w2p.tile([128, 16, D], BF16, name="w2T_sb")
                    for c in range(0, 16, 4):
                        dma(out=w2T_sb[:, c:c + 4, :],
                            in_=w2T_d.ap().rearrange("(c p) m -> p c m",
                                                     p=128)[:, c:c + 4, :])
                    hT_sb = hpool.tile([128, 16, SQ], BF16, name="hT_sb")

                    def ffn1_m(m):
                        f_ps = ps_tile(f"f_ps{m}")
                        for tj in range(2):
                            for cp in range(2):
                                nc.tensor.matmul(
                                    f_ps[:, 512 * tj:512 * tj + 512],
                                    w1T_sb[:, 2 * cp:2 * cp + 2,
                                           128 * m:128 * m + 128],
                                    actT8_sb[:, 2 * cp:2 * cp + 2,
                                             512 * tj:512 * tj + 512],
                                    start=(cp == 0), stop=(cp == 1),
                                    perf_mode=PM.DoubleRow)
                        nc.scalar.activation(out=hT_sb[:, m, :], in_=f_ps[:],
                                             func=AF.Gelu,
                                             bias=b1_sb[:, m:m + 1], scale=1.0)

                    def ffn2_fc(y_ps, wave, fc):
                        for jj in range(4):
                            j = 4 * wave + jj
                            nc.tensor.matmul(
                                y_ps[:, 512 * jj:512 * jj + 512],
                                hT_sb[:, fc, 128 * j:128 * j + 128],
                                w2T_sb[:, fc, :],
                                start=(fc == 0), stop=(fc == 15))

                    def ln2_wave(y_ps, wave):
                        for jj in range(4):
                            j = 4 * wave + jj
                            o_sb = work.tile([128, D], F32, tag="o",
                                             name=f"o{j}")
                            layer_norm(8 + j, y_ps[:, 512 * jj:512 * jj + 512],
                                       xown_sb[:, j, :], o_sb[:],
                                       bc.get("b2"), bc.get("g2"),
                                       bc.get("bt2"))
                            dma(out=out_d[128 * j:128 * j + 128, :],
                                in_=o_sb[:])

                    for m in range(16):
                        ffn1_m(m)
                    if stop_after == "ffn1":
                        return
                    # plain per-token-block FFN2: each block's 16-chunk psum
                    # accumulation chases the gelu stream naturally (fc<m
                    # chunks run while later gelus are still in flight)
                    for j in range(8):
                        y_ps = ps_tile(f"y_ps{j}")
                        for fc in range(16):
                            nc.tensor.matmul(y_ps[:, 0:512],
                                             hT_sb[:, fc, 128 * j:128 * j + 128],
                                             w2T_sb[:, fc, :],
                                             start=(fc == 0), stop=(fc == 15))
                        o_sb = work.tile([128, D], F32, tag="o", name=f"o{j}")
                        layer_norm(8 + j, y_ps[:, 0:512], xown_sb[:, j, :],
                                   o_sb[:],
                                   bc.get("b2"), bc.get("g2"), bc.get("bt2"))
                        dma(out=out_d[128 * j:128 * j + 128, :], in_=o_sb[:])

    # KERNEL_UNROLL: bodies per For_i iteration. The loop's per-iteration
    # all-engine barrier + semaphore reset is measurement scaffolding, not
    # kernel work; unrolling amortizes it so the measured slope approaches
    # the true back-to-back kernel time.
    unroll = int(os.environ.get("KERNEL_UNROLL", "1"))
    with tile.TileContext(nc) as tc:
        if reps == 1:
            body(tc)
        else:
            assert reps % unroll == 0, (reps, unroll)
            with tc.For_i(0, reps // unroll):
                for _u in range(unroll):
                    body(tc)
    return nc


_NC_CACHE = {}


def _get_nc(flags):
    key = (tuple(sorted(flags.items())),
           os.environ.get("KERNEL_STOP_AFTER", ""),
           os.environ.get("KERNEL_REPS", "1"),
           os.environ.get("KERNEL_UNROLL", "1"))
    if key not in _NC_CACHE:
        nc = bacc.Bacc("TRN2", target_bir_lowering=False, debug=False)
        _emit(nc, flags)
        nc.compile()
        _NC_CACHE[key] = nc
    return _NC_CACHE[key]


LAST_RESULTS = None


def make_in_maps(x, in_proj_w, in_proj_b, out_w, out_b, ln1_g, ln1_b, ln2_g,
                 ln2_b, ff_w1, ff_b1, ff_w2, ff_b2):
    x = np.asarray(x, dtype=np.float32)
    scale = np.float32(1.0 / np.sqrt(HD))

    wqkvT_f = np.ascontiguousarray(np.asarray(in_proj_w, np.float32).T)  # (D, 3D)
    wqkvT_f[:, :D] *= scale
    wqk8 = np.ascontiguousarray(wqkvT_f[:, :2 * D]).astype(
        ml_dtypes.float8_e4m3fn)
    wv8 = np.ascontiguousarray(wqkvT_f[:, 2 * D:]).astype(ml_dtypes.float8_e4m3fn)
    bqkv = np.asarray(in_proj_b, np.float32).copy()
    bqkv[:D] *= scale
    bqkv_pp = np.ascontiguousarray(bqkv.reshape(12, 128).T)
    woutT = np.ascontiguousarray(np.asarray(out_w, np.float32).T).astype(
        ml_dtypes.bfloat16)
    w1T = np.ascontiguousarray(np.asarray(ff_w1, np.float32).T).astype(
        ml_dtypes.float8_e4m3fn)
    b1_pp = np.ascontiguousarray(np.asarray(ff_b1, np.float32).reshape(16, 128).T)
    w2T = np.ascontiguousarray(np.asarray(ff_w2, np.float32).T).astype(
        ml_dtypes.bfloat16)

    assign = np.zeros((8, 4, 128), np.float32)
    for h in range(8):
        i = h // 2
        lo = 64 * (h % 2)
        assign[h, i, lo:lo + 64] = 1.0
    ident = np.eye(128, dtype=np.float32)

    bv = bqkv[2 * D:3 * D]
    vecs = np.stack([
        bv,
        np.asarray(out_b, np.float32),
        np.asarray(ff_b2, np.float32),
        np.asarray(ln1_g, np.float32),
        np.asarray(ln1_b, np.float32),
        np.asarray(ln2_g, np.float32),
        np.asarray(ln2_b, np.float32),
    ]).astype(np.float32)

    flags = {
        "bv": bool(np.any(bv != 0)),
        "bout": bool(np.any(vecs[1] != 0)),
        "b2": bool(np.any(vecs[2] != 0)),
        "g1": bool(np.any(vecs[3] != 1)),
        "bt1": bool(np.any(vecs[4] != 0)),
        "g2": bool(np.any(vecs[5] != 1)),
        "bt2": bool(np.any(vecs[6] != 0)),
    }

    in_maps = []
    for c in range(N_CORES):
        b, hh = c // 2, c % 2
        xb = x[b]
        xT = np.ascontiguousarray(xb.T) if hh == 0 else \
            np.ascontiguousarray(np.roll(xb.T, -SQ, axis=1))
        in_maps.append({
            "xT8": xT.astype(ml_dtypes.float8_e4m3fn), "wv8": wv8,
            "wqk8": wqk8,
            "x_own": np.ascontiguousarray(xb[SQ * hh:SQ * (hh + 1)]),
            "bqkv_pp": bqkv_pp, "woutT": woutT,
            "w1T": w1T, "b1_pp": b1_pp, "w2T": w2T,
            "assign": assign, "ident": ident, "vecs": vecs,
        })
    return in_maps, flags


def kernel(x, in_proj_w, in_proj_b, out_w, out_b, ln1_g, ln1_b, ln2_g, ln2_b,
           ff_w1, ff_b1, ff_w2, ff_b2):
    global LAST_RESULTS
    in_maps, flags = make_in_maps(
        x, in_proj_w, in_proj_b, out_w, out_b, ln1_g, ln1_b, ln2_g, ln2_b,
        ff_w1, ff_b1, ff_w2, ff_b2)
    nc = _get_nc(flags)
    res = run_bass_kernel_spmd(
        nc, in_maps, core_ids=list(range(N_CORES)),
        trace=bool(int(os.environ.get("BASS_KERNEL_TRACE", "0"))))
    LAST_RESULTS = res

    out = np.empty((B, S, D), np.float32)
    for c in range(N_CORES):
        b, hh = c // 2, c % 2
        out[b, SQ * hh:SQ * (hh + 1)] = res.results[c]["out"]
    return out

